# revision 25
# baseline (speedup 1.0000x reference)
"""Transformer kernel builder for TRN2 (Bass/Tile), data-parallel over batch.

Per-core: 2 batch elements (T=1024 tokens), full weights.
Feature-major activations [D, T]; bf16 matmuls; fp16 FFN hidden + W2.
"""
import numpy as np
from contextlib import ExitStack

import concourse.bass as bass
import concourse.bacc as bacc
import concourse.tile as tile
from concourse import mybir
from concourse.masks import make_identity

P = 128
S = 512
BL = 2            # local batches per core
T = S * BL        # 1024 tokens per core
D = 1024
H = 16
DK = 64
DHID = 4096
DOUT = 10000
L = 4
LN_EPS = 1e-5
MASK_RATE = 0.15
NDC = D // P      # 8 d-chunks
NHC = DHID // P   # 32 hid chunks
NOC = 20          # dout chunks of 512 (last 272)

f32 = mybir.dt.float32
f32r = mybir.dt.float32r
f16 = mybir.dt.float16
bf16 = mybir.dt.bfloat16
AF = mybir.ActivationFunctionType
OP = mybir.AluOpType

UW = 2048         # unit width in fp32 elements (8 KiB slots)


_name_ctr = [0]


def _nm(prefix):
    _name_ctr[0] += 1
    return f"{prefix}{_name_ctr[0]}"


class FM:
    """Chunked buffer: nch chunks of [128, ncols], packed into 8 KiB units."""

    def __init__(self, pool, nch, ncols, dtype):
        self.nch, self.ncols = nch, ncols
        uw = UW * (2 if dtype in (f16, bf16) else 1)
        self.cpu = max(1, uw // ncols)
        n_units = (nch + self.cpu - 1) // self.cpu
        self.units = [pool.tile([P, self.cpu * ncols], dtype, tag="u",
                                name=_nm("fm"))
                      for _ in range(n_units)]

    def sl(self, dc, c0=0, n=None, p0=0, np_=P):
        n = self.ncols - c0 if n is None else n
        u = self.units[dc // self.cpu]
        base = (dc % self.cpu) * self.ncols
        return u[p0:p0 + np_, base + c0: base + c0 + n]


def build(n_layers=L, do_final=True, dumps=(), n_cores=8, u_bufs=16):
    nc = bacc.Bacc("TRN2", target_bir_lowering=False, debug=False,
                   num_devices=n_cores)
    dp = nc.declare_dram_parameter
    xb = dp("xb", [S, BL, D], f32, isOutput=False)
    rnd = dp("rnd", [BL, S], f32, isOutput=False)
    posi = dp("posi", [S, D], f32, isOutput=False)
    ln0w = dp("ln0_w", [D], f32, isOutput=False)
    ln0b = dp("ln0_b", [D], f32, isOutput=False)
    WqT = dp("WqT", [L, D, D], bf16, isOutput=False)
    WkT = dp("WkT", [L, D, D], bf16, isOutput=False)
    WvT = dp("WvT", [L, D, D], bf16, isOutput=False)
    WfcT = dp("WfcT", [L, D, D], bf16, isOutput=False)
    W1T = dp("W1T", [L, D, DHID], bf16, isOutput=False)
    W2T = dp("W2T", [L, DHID, D], f16, isOutput=False)
    bqc = dp("bqc", [L, P, NDC], f32, isOutput=False)
    bkc = dp("bkc", [L, P, NDC], f32, isOutput=False)
    bfcc = dp("bfcc", [L, P, NDC], f32, isOutput=False)
    b1c = dp("b1c", [L, P, NHC], f32, isOutput=False)
    b2c = dp("b2c", [L, P, NDC], f32, isOutput=False)
    l1wc = dp("l1wc", [L, P, NDC], f32, isOutput=False)
    l1bc = dp("l1bc", [L, P, NDC], f32, isOutput=False)
    l2wc = dp("l2wc", [L, P, NDC], f32, isOutput=False)
    l2bc = dp("l2bc", [L, P, NDC], f32, isOutput=False)
    bv = dp("bv", [L, D], f32, isOutput=False)
    WoT = dp("WoT", [D, DOUT], bf16, isOutput=False)
    bo = dp("bo", [DOUT], f32, isOutput=False)
    out = dp("out", [S, BL, DOUT], f32, isOutput=True) if do_final else None
    dump_t = {}

    def dump_fm(nm, fm):
        if nm not in dumps:
            return
        w = fm.units[0].shape[1]
        dt_ = fm.units[0].dtype
        dump_t[nm] = dp("dump_" + nm, [len(fm.units), P, w], dt_, isOutput=True)
        for i, u in enumerate(fm.units):
            nc.sync.dma_start(dump_t[nm][i], u[:])

    with tile.TileContext(nc) as tc:
        with ExitStack() as ctx:
            ctx.enter_context(nc.allow_low_precision(
                "f32r/f16 matmul operands by design; accumulation is f32"))
            pu = ctx.enter_context(tc.tile_pool(name="pu", bufs=u_bufs))
            pw = ctx.enter_context(tc.tile_pool(name="pw", bufs=6))
            pwl = ctx.enter_context(tc.tile_pool(name="pwl", bufs=8))
            pwr = ctx.enter_context(tc.tile_pool(name="pwr", bufs=4))
            pb = ctx.enter_context(tc.tile_pool(name="pb", bufs=10))
            pbv = ctx.enter_context(tc.tile_pool(name="pbv", bufs=1))
            pr = ctx.enter_context(tc.tile_pool(name="pr", bufs=6))
            pst = ctx.enter_context(tc.tile_pool(name="pst", bufs=8))
            pc = ctx.enter_context(tc.tile_pool(name="pc", bufs=1))
            ps = ctx.enter_context(tc.tile_pool(name="ps", bufs=8, space="PSUM"))

            # ---- constants ----
            ident = pc.tile([P, P], f32, tag="c_id")
            make_identity(nc, ident[:])
            ones_f = pc.tile([P, 1], f32, tag="c_of")
            nc.vector.memset(ones_f[:], 1.0)
            ones_col = pc.tile([P, 1], bf16, tag="c_oc")
            nc.vector.tensor_copy(ones_col[:], ones_f[:])
            ones_rf = pc.tile([1, P], f32, tag="c_orf")
            nc.vector.memset(ones_rf[:], 1.0)
            ones_row = pc.tile([1, P], bf16, tag="c_or")
            nc.vector.tensor_copy(ones_row[:], ones_rf[:])
            lnw_rep = pc.tile([P, D], f32, tag="c_lnw")
            nc.sync.dma_start(lnw_rep[:], ln0w[:].rearrange("(o d) -> o d", o=1)
                              .to_broadcast((P, D)))
            lnb_rep = pc.tile([P, D], f32, tag="c_lnb")
            eps_col = pc.tile([P, 1], f32, tag="c_eps")
            nc.vector.memset(eps_col[:], LN_EPS)
            nc.sync.dma_start(lnb_rep[:], ln0b[:].rearrange("(o d) -> o d", o=1)
                              .to_broadcast((P, D)))

            def psum(shape=(P, 512), dtype=f32):
                return ps.tile(list(shape), dtype, tag="ps", name=_nm("ps"))

            # ================= embed =================
            posib = FM(pu, 4, 1024, f32)  # s-chunk major
            for scj in range(4):
                pt = posib.sl(scj)
                nc.sync.dma_start(pt, posi[scj * P:(scj + 1) * P, :])
                nc.vector.tensor_tensor(out=pt, in0=pt, in1=lnb_rep[:], op=OP.add)

            enc = FM(pu, NDC, 1024, bf16)
            for b in range(BL):
                for sc in range(4):
                    x_tm = pw.tile([P, D], f32, tag="w")
                    nc.sync.dma_start(x_tm[:], xb[sc * P:(sc + 1) * P, b, :])
                    kcol = pst.tile([P, 1], f32, tag="st")
                    nc.sync.dma_start(
                        kcol[:], rnd[b, sc * P:(sc + 1) * P]
                        .rearrange("(p o) -> p o", o=1))
                    km = pst.tile([P, 1], f32, tag="st")
                    nc.vector.tensor_scalar(out=km[:], in0=kcol[:],
                                            scalar1=MASK_RATE, scalar2=None,
                                            op0=OP.is_gt)
                    h = pw.tile([P, D], f32, tag="w")
                    nc.vector.tensor_scalar(out=h[:], in0=x_tm[:], scalar1=km[:],
                                            scalar2=None, op0=OP.mult)
                    stats = pst.tile([P, 2, 6], f32, tag="st6")
                    hr = h[:].rearrange("p (g f) -> p g f", g=2)
                    for g in range(2):
                        nc.vector.bn_stats(out=stats[:, g, :], in_=hr[:, g, :])
                    mv = pst.tile([P, 2], f32, tag="st")
                    nc.vector.bn_aggr(out=mv[:], in_=stats[:])
                    sd = pst.tile([P, 1], f32, tag="st")
                    nc.scalar.activation(out=sd[:], in_=mv[:, 1:2], func=AF.Sqrt,
                                         bias=eps_col[:])
                    rs = pst.tile([P, 1], f32, tag="st")
                    nc.vector.reciprocal(out=rs[:], in_=sd[:])
                    t1 = pw.tile([P, D], f32, tag="w")
                    nc.vector.scalar_tensor_tensor(
                        out=t1[:], in0=h[:], scalar=mv[:, 0:1],
                        in1=rs[:].to_broadcast((P, D)),
                        op0=OP.subtract, op1=OP.mult)
                    nc.vector.tensor_tensor(out=t1[:], in0=t1[:], in1=lnw_rep[:],
                                            op=OP.mult)
                    nc.vector.tensor_tensor(out=t1[:], in0=t1[:],
                                            in1=posib.sl(sc), op=OP.add)
                    for j in range(NDC):
                        pt = psum((P, P))
                        nc.tensor.transpose(pt[:, :], t1[:, j * P:(j + 1) * P],
                                            ident[:])
                        nc.vector.tensor_copy(
                            enc.sl(j, b * S + sc * P, P), pt[:, :])

            dump_fm("enc0", enc)

            # ================= helpers =================
            def load_cols(src, l, n):
                t = pb.tile([P, n], f32, tag="b")
                nc.sync.dma_start(t[:], src[l])
                return t

            def ln_fm(X, c0_in, n, w_t, b_t, dst, c0_out):
                """LN over the feature (partition-chunk) dim on columns
                [c0_in, c0_in+n) of X, writing [c0_out, c0_out+n) of dst."""
                mu_ps = psum((1, n))
                sq_ps = psum((1, n))
                for dc in range(NDC):
                    xs = X.sl(dc, c0_in, n)
                    nc.tensor.matmul(mu_ps[:, :], ones_col[:], xs,
                                     start=(dc == 0), stop=(dc == NDC - 1))
                    sq = pw.tile([P, n], bf16, tag="w")
                    nc.vector.tensor_tensor(out=sq[:], in0=xs, in1=xs, op=OP.mult)
                    nc.tensor.matmul(sq_ps[:, :], ones_col[:], sq[:],
                                     start=(dc == 0), stop=(dc == NDC - 1))
                mu = pr.tile([1, n], f32, tag="r")
                nc.vector.tensor_scalar(out=mu[:], in0=mu_ps[:, :],
                                        scalar1=1.0 / D, scalar2=None, op0=OP.mult)
                mu2 = pr.tile([1, n], f32, tag="r")
                nc.vector.tensor_tensor(out=mu2[:], in0=mu[:], in1=mu[:],
                                        op=OP.mult)
                var = pr.tile([1, n], f32, tag="r")
                nc.vector.scalar_tensor_tensor(
                    out=var[:], in0=sq_ps[:, :], scalar=1.0 / D, in1=mu2[:],
                    op0=OP.mult, op1=OP.subtract)
                nc.scalar.activation(out=var[:], in_=var[:], func=AF.Sqrt,
                                     bias=eps_col[0:1, :])
                g_r = pr.tile([1, n], bf16, tag="r")
                nc.vector.reciprocal(out=g_r[:], in_=var[:])
                c_r = pr.tile([1, n], bf16, tag="r")
                nc.vector.tensor_tensor(out=c_r[:], in0=mu[:], in1=g_r[:],
                                        op=OP.mult)
                g_ps = psum((P, n))
                nc.tensor.matmul(g_ps[:, :], ones_row[:], g_r[:],
                                 start=True, stop=True)
                c_ps = psum((P, n))
                nc.tensor.matmul(c_ps[:, :], ones_row[:], c_r[:],
                                 start=True, stop=True)
                for dc in range(NDC):
                    t1 = pw.tile([P, n], f32, tag="w")
                    nc.vector.tensor_tensor(out=t1[:], in0=X.sl(dc, c0_in, n),
                                            in1=g_ps[:, :], op=OP.mult)
                    nc.vector.tensor_tensor(out=t1[:], in0=t1[:],
                                            in1=c_ps[:, :], op=OP.subtract)
                    nc.vector.scalar_tensor_tensor(
                        out=dst.sl(dc, c0_out, n), in0=t1[:],
                        scalar=w_t[:, dc:dc + 1],
                        in1=b_t[:, dc:dc + 1].to_broadcast((P, n)),
                        op0=OP.mult, op1=OP.add)

            # ================= layers =================
            for l in range(n_layers):
                bqt = load_cols(bqc, l, NDC)
                bkt = load_cols(bkc, l, NDC)
                bfct = load_cols(bfcc, l, NDC)
                b1t = load_cols(b1c, l, NHC)
                b2t = load_cols(b2c, l, NDC)
                l1wt = load_cols(l1wc, l, NDC)
                l1bt = load_cols(l1bc, l, NDC)
                l2wt = load_cols(l2wc, l, NDC)
                l2bt = load_cols(l2bc, l, NDC)
                bvrep = pbv.tile([P, D], f32, tag="bv")
                nc.sync.dma_start(bvrep[:], bv[l].rearrange("(o d) -> o d", o=1)
                                  .to_broadcast((P, D)))

                # ---- q/k projections, both batches, weights once ----
                qk = {}
                for (nm, W, bt) in (("q", WqT, bqt), ("k", WkT, bkt)):
                    for b in range(BL):
                        qk[(nm, b)] = FM(pu, NDC, S, bf16)
                    for m in range(NDC):
                        pps = [psum(), psum()]
                        for k in range(NDC):
                            wt = pwl.tile([P, P], bf16, tag="wl")
                            nc.sync.dma_start(
                                wt[:], W[l, k * P:(k + 1) * P, m * P:(m + 1) * P])
                            for b in range(BL):
                                nc.tensor.matmul(
                                    pps[b][:, :], wt[:], enc.sl(k, b * S, S),
                                    start=(k == 0), stop=(k == NDC - 1))
                        for b in range(BL):
                            nc.scalar.activation(
                                out=qk[(nm, b)].sl(m), in_=pps[b][:, :],
                                func=AF.Identity, bias=bt[:, m:m + 1])

                if l == 0:
                    dump_fm("q0", qk[("q", 0)])
                    dump_fm("k0", qk[("k", 0)])

                C = [None, None]
                for b in range(BL):
                    qb, kb = qk[("q", b)], qk[("k", b)]
                    # ---- v projection (token-major) for batch b ----
                    vT = FM(pu, 4, 1024, bf16)  # [512 tok, 1024 feat]
                    for n in range(2):
                        pps = [psum() for _ in range(4)]
                        for k in range(NDC):
                            wt = pwr.tile([P, 512], bf16, tag="wr")
                            nc.sync.dma_start(
                                wt[:], WvT[l, k * P:(k + 1) * P,
                                           n * 512:(n + 1) * 512])
                            for tcc in range(4):
                                nc.tensor.matmul(
                                    pps[tcc][:, :],
                                    enc.sl(k, b * S + tcc * P, P), wt[:],
                                    start=(k == 0), stop=(k == NDC - 1))
                        for tcc in range(4):
                            nc.vector.tensor_tensor(
                                out=vT.sl(tcc, n * 512, 512),
                                in0=pps[tcc][:, :],
                                in1=bvrep[:, n * 512:(n + 1) * 512], op=OP.add)
                    if l == 0 and b == 0:
                        dump_fm("v0", vT)
                    # ---- attention for batch b ----
                    att = FM(pu, NDC, S, bf16)
                    for h in range(H):
                        dc = h // 2
                        po = (h % 2) * DK
                        exps = []
                        den_ps = psum((1, S))
                        for kc in range(4):
                            sc_ps = psum()
                            nc.tensor.matmul(
                                sc_ps[:, :],
                                kb.sl(dc, kc * P, P, p0=po, np_=DK),
                                qb.sl(dc, 0, S, p0=po, np_=DK),
                                start=True, stop=True)
                            ex = pw.tile([P, S], bf16, tag="w")
                            nc.scalar.activation(out=ex[:], in_=sc_ps[:, :],
                                                 func=AF.Exp, scale=0.125)
                            exps.append(ex)
                            nc.tensor.matmul(den_ps[:, :], ones_col[:], ex[:],
                                             start=(kc == 0), stop=(kc == 3))
                        av_ps = psum((DK, S))
                        for kc in range(4):
                            nc.tensor.matmul(
                                av_ps[:, :], vT.sl(kc, h * DK, DK), exps[kc][:],
                                start=(kc == 0), stop=(kc == 3))
                        den_r = pr.tile([1, S], bf16, tag="r")
                        nc.vector.reciprocal(out=den_r[:], in_=den_ps[:, :])
                        rep_ps = psum((DK, S))
                        nc.tensor.matmul(rep_ps[:, :], ones_row[:, 0:DK],
                                         den_r[:], start=True, stop=True)
                        asl = att.sl(dc, 0, S, p0=po, np_=DK)
                        nc.vector.tensor_copy(asl, av_ps[:, :])
                        nc.vector.tensor_tensor(out=asl, in0=asl,
                                                in1=rep_ps[:, :], op=OP.mult)
                    if l == 0 and b == 0:
                        dump_fm("att0", att)
                    # ---- fc + bias + residual for batch b ----
                    C[b] = FM(pu, NDC, S, bf16)
                    for m in range(NDC):
                        pp = psum()
                        for k in range(NDC):
                            wt = pwl.tile([P, P], bf16, tag="wl")
                            nc.sync.dma_start(
                                wt[:], WfcT[l, k * P:(k + 1) * P,
                                            m * P:(m + 1) * P])
                            nc.tensor.matmul(pp[:, :], wt[:], att.sl(k),
                                             start=(k == 0), stop=(k == NDC - 1))
                        nc.vector.scalar_tensor_tensor(
                            out=C[b].sl(m), in0=pp[:, :],
                            scalar=bfct[:, m:m + 1], in1=enc.sl(m, b * S, S),
                            op0=OP.add, op1=OP.add)
                if l == 0:
                    dump_fm("c0", C[0])

                # ---- LN1 -> Dm ----
                Dm = FM(pu, NDC, 1024, bf16)
                for b in range(BL):
                    ln_fm(C[b], 0, S, l1wt, l1bt, Dm, b * S)

                # ---- FFN (both halves, weights once) ----
                hid = FM(pu, NHC, 1024, f16)
                for m in range(NHC):
                    pps = [psum(), psum()]
                    for k in range(NDC):
                        wt = pwl.tile([P, P], bf16, tag="wl")
                        nc.sync.dma_start(
                            wt[:], W1T[l, k * P:(k + 1) * P, m * P:(m + 1) * P])
                        for th in range(2):
                            nc.tensor.matmul(
                                pps[th][:, :], wt[:], Dm.sl(k, th * S, S),
                                start=(k == 0), stop=(k == NDC - 1))
                    for th in range(2):
                        nc.scalar.activation(
                            out=hid.sl(m, th * S, S), in_=pps[th][:, :],
                            func=AF.Relu, bias=b1t[:, m:m + 1])
                E = FM(pu, NDC, 1024, bf16)
                for m in range(NDC):
                    pps = [psum(), psum()]
                    for k in range(NHC):
                        wt = pwl.tile([P, P], f16, tag="wl")
                        nc.sync.dma_start(
                            wt[:], W2T[l, k * P:(k + 1) * P, m * P:(m + 1) * P])
                        for th in range(2):
                            nc.tensor.matmul(
                                pps[th][:, :], wt[:], hid.sl(k, th * S, S),
                                start=(k == 0), stop=(k == NHC - 1))
                    for th in range(2):
                        nc.vector.scalar_tensor_tensor(
                            out=E.sl(m, th * S, S), in0=pps[th][:, :],
                            scalar=b2t[:, m:m + 1], in1=Dm.sl(m, th * S, S),
                            op0=OP.add, op1=OP.add)

                # ---- LN2 -> next enc ----
                F = FM(pu, NDC, 1024, bf16)
                for th in range(2):
                    ln_fm(E, th * S, S, l2wt, l2bt, F, th * S)
                enc = F
                dump_fm(f"enc_l{l}", enc)

            # ================= final projection + log_softmax =================
            if do_final:
                for tg in range(2):
                    lgs = [[pu.tile([P, 4096], f16, tag="u", name=_nm("lg")) for _ in range(3)]
                           for _ in range(4)]
                    zaccs = [pst.tile([P, NOC], f32, tag="z", name=_nm("za")) for _ in range(4)]
                    for n in range(NOC):
                        ncols = 512 if n < NOC - 1 else DOUT - (NOC - 1) * 512
                        borep = pw.tile([P, 512], f32, tag="w")
                        nc.sync.dma_start(
                            borep[:, :ncols],
                            bo[n * 512:n * 512 + ncols]
                            .rearrange("(o d) -> o d", o=1)
                            .to_broadcast((P, ncols)))
                        pps = [psum() for _ in range(4)]
                        for k in range(NDC):
                            wt = pwr.tile([P, 512], bf16, tag="wr")
                            nc.sync.dma_start(
                                wt[:, :ncols],
                                WoT[k * P:(k + 1) * P, n * 512:n * 512 + ncols])
                            for tcc in range(4):
                                nc.tensor.matmul(
                                    pps[tcc][:, :ncols],
                                    enc.sl(k, tg * S + tcc * P, P),
                                    wt[:, :ncols],
                                    start=(k == 0), stop=(k == NDC - 1))
                        for tcc in range(4):
                            lsl = lgs[tcc][n // 8][:, (n % 8) * 512:
                                                   (n % 8) * 512 + ncols]
                            nc.vector.tensor_tensor(out=lsl, in0=pps[tcc][:, :ncols],
                                                    in1=borep[:, :ncols], op=OP.add)
                            exs = pw.tile([P, 512], f16, tag="w")
                            nc.scalar.activation(
                                out=exs[:, :ncols], in_=lsl, func=AF.Exp,
                                accum_out=zaccs[tcc][:, n:n + 1])
                    for tcc in range(4):
                        z = pst.tile([P, 1], f32, tag="st")
                        nc.vector.reduce_sum(z[:], zaccs[tcc][:],
                                             axis=mybir.AxisListType.X)
                        lz = pst.tile([P, 1], f32, tag="st")
                        nc.scalar.activation(out=lz[:], in_=z[:], func=AF.Ln)
                        for n in range(NOC):
                            ncols = 512 if n < NOC - 1 else DOUT - (NOC - 1) * 512
                            lsl = lgs[tcc][n // 8][:, (n % 8) * 512:
                                                   (n % 8) * 512 + ncols]
                            st = pw.tile([P, 512], f32, tag="w")
                            nc.vector.tensor_scalar(
                                out=st[:, :ncols], in0=lsl, scalar1=lz[:],
                                scalar2=None, op0=OP.subtract)
                            s0 = tcc * P
                            nc.sync.dma_start(
                                out[s0:s0 + P, tg, n * 512:n * 512 + ncols],
                                st[:, :ncols])
    nc.finalize()
    return nc


# ======================= host-side input prep =======================
def make_in_map(inp, core):
    """Build the per-core input dict from the full-problem input dict."""
    import ml_dtypes
    bf = ml_dtypes.bfloat16
    f = np.float32
    c = np.ascontiguousarray
    b0 = core * BL
    m = {
        "xb": c(np.asarray(inp["x"], f)[:, b0:b0 + BL, :]),
        "rnd": c(np.asarray(inp["rnd"], f)[b0:b0 + BL, :]),
        "posi": c(np.asarray(inp["posi"], f)),
        "ln0_w": c(np.asarray(inp["ln0_w"], f)),
        "ln0_b": c(np.asarray(inp["ln0_b"], f)),
        "bv": c(np.asarray(inp["bv"], f)),
        "bo": c(np.asarray(inp["bo"], f)),
    }
    tr = lambda a: c(np.asarray(a, f).transpose(0, 2, 1).astype(bf))
    m["WqT"] = tr(inp["Wq"])
    m["WkT"] = tr(inp["Wk"])
    m["WvT"] = tr(inp["Wv"])
    m["WfcT"] = tr(inp["Wfc"])
    m["W1T"] = tr(inp["W1"])
    m["W2T"] = np.asarray(inp["W2"], f).transpose(0, 2, 1).astype(np.float16)
    m["W2T"] = c(m["W2T"])
    m["WoT"] = c(np.asarray(inp["Wo"], f).T.astype(bf))
    cols = lambda a, nch: c(np.asarray(a, f).reshape(L, nch, P).transpose(0, 2, 1))
    m["bqc"] = cols(inp["bq"], NDC)
    m["bkc"] = cols(inp["bk"], NDC)
    m["bfcc"] = cols(inp["bfc"], NDC)
    m["b1c"] = cols(inp["b1"], NHC)
    m["b2c"] = cols(inp["b2"], NDC)
    m["l1wc"] = cols(inp["ln1_w"], NDC)
    m["l1bc"] = cols(inp["ln1_b"], NDC)
    m["l2wc"] = cols(inp["ln2_w"], NDC)
    m["l2bc"] = cols(inp["ln2_b"], NDC)
    return m


def fm_to_np(arr, nch, ncols, dtype_bytes=4):
    """[n_units, 128, unit_cols] -> [nch*128, ncols]."""
    n_units = arr.shape[0]
    uw = arr.shape[2]
    cpu = uw // ncols
    out = np.zeros((nch * P, ncols), arr.dtype)
    for dc in range(nch):
        u = arr[dc // cpu]
        base = (dc % cpu) * ncols
        out[dc * P:(dc + 1) * P, :] = u[:, base:base + ncols]
    return out


# ======================= entry point =======================
_NC_CACHE = {}


def _get_nc(n_cores=8):
    if n_cores not in _NC_CACHE:
        _NC_CACHE[n_cores] = build(n_layers=L, do_final=True, dumps=(),
                                   n_cores=n_cores)
    return _NC_CACHE[n_cores]


def kernel(**inputs):
    """Full-input, full-output entry point. Shards batch across 8 cores."""
    from concourse.bass_utils import run_bass_kernel_spmd
    n_cores = 8
    nc = _get_nc(n_cores)
    inp = {k: np.asarray(v) for k, v in inputs.items()}
    in_maps = [make_in_map(inp, c) for c in range(n_cores)]
    res = run_bass_kernel_spmd(nc, in_maps, list(range(n_cores)))
    outs = [res.results[c]["out"] for c in range(n_cores)]
    return np.concatenate(outs, axis=1).astype(np.float32)



# revision 51
# speedup vs baseline: 1.6922x; 1.6922x over previous
"""Transformer kernel builder for TRN2 (Bass/Tile), data-parallel over batch.

Per-core: 2 batch elements (T=1024 tokens), full weights.
Feature-major activations [D, T]; bf16 matmuls; fp8 FFN hidden + W2.
"""
import numpy as np
from contextlib import ExitStack

import concourse.bass as bass
import concourse.bacc as bacc
import concourse.tile as tile
from concourse import mybir
from concourse.masks import make_identity

P = 128
S = 512
BL = 2            # local batches per core
T = S * BL        # 1024 tokens per core
D = 1024
H = 16
DK = 64
DHID = 4096
DOUT = 10000
L = 4
LN_EPS = 1e-5
MASK_RATE = 0.15
NDC = D // P      # 8 d-chunks
NHC = DHID // P   # 32 hid chunks
NOC = 20          # dout chunks of 512 (last 272)
W2_SCALE = 64.0   # host scales W2 by this; descaled in the bias activation

f32 = mybir.dt.float32
f16 = mybir.dt.float16
bf16 = mybir.dt.bfloat16
f8 = mybir.dt.float8e4
AF = mybir.ActivationFunctionType
OP = mybir.AluOpType

UW = 2048         # unit width in fp32 elements (8 KiB slots)


_name_ctr = [0]


def _nm(prefix):
    _name_ctr[0] += 1
    return f"{prefix}{_name_ctr[0]}"


def _dtw(dtype):
    return 2 if dtype in (f16, bf16) else (4 if dtype == f8 else 1)


class FM:
    """Chunked buffer: nch chunks of [128, ncols], packed into 8 KiB units."""

    def __init__(self, pool, nch, ncols, dtype):
        self.nch, self.ncols = nch, ncols
        uw = UW * _dtw(dtype)
        self.cpu = max(1, uw // ncols)
        n_units = (nch + self.cpu - 1) // self.cpu
        self.units = [pool.tile([P, self.cpu * ncols], dtype, tag="u",
                                name=_nm("fm"))
                      for _ in range(n_units)]

    def sl(self, dc, c0=0, n=None, p0=0, np_=P):
        n = self.ncols - c0 if n is None else n
        u = self.units[dc // self.cpu]
        base = (dc % self.cpu) * self.ncols
        return u[p0:p0 + np_, base + c0: base + c0 + n]


def build(n_layers=L, do_final=True, dumps=(), n_cores=8, u_bufs=14):
    nc = bacc.Bacc("TRN2", target_bir_lowering=False, debug=False,
                   num_devices=n_cores)
    dp = nc.declare_dram_parameter
    xb = dp("xb", [S, BL, D], f32, isOutput=False)
    rnd = dp("rnd", [BL, S], f32, isOutput=False)
    posib_d = dp("posib", [S, D], f32, isOutput=False)  # posi + ln0_b (host)
    ln0w = dp("ln0_w", [D], f32, isOutput=False)
    WqT = dp("WqT", [L, D, D], bf16, isOutput=False)
    WkT = dp("WkT", [L, D, D], bf16, isOutput=False)
    WvT = dp("WvT", [L, D, D], bf16, isOutput=False)
    WfcT = dp("WfcT", [L, D, D], bf16, isOutput=False)
    W1T = dp("W1T", [L, D, DHID], bf16, isOutput=False)
    W2T = dp("W2T", [L, DHID, D], f8, isOutput=False)
    bqc = dp("bqc", [L, P, NDC], f32, isOutput=False)
    bkc = dp("bkc", [L, P, NDC], f32, isOutput=False)
    bfcc = dp("bfcc", [L, P, NDC], f32, isOutput=False)
    b1c = dp("b1c", [L, P, NHC], f32, isOutput=False)
    b2c = dp("b2c", [L, P, NDC], f32, isOutput=False)
    l1wc = dp("l1wc", [L, P, NDC], f32, isOutput=False)
    l1bc = dp("l1bc", [L, P, NDC], f32, isOutput=False)
    l2wc = dp("l2wc", [L, P, NDC], f32, isOutput=False)
    l2bc = dp("l2bc", [L, P, NDC], f32, isOutput=False)
    bv = dp("bv", [L, D], f32, isOutput=False)
    WoT = dp("WoT", [D, DOUT], bf16, isOutput=False)
    bo = dp("bo", [DOUT], bf16, isOutput=False)
    out = dp("out", [S, BL, DOUT], f16, isOutput=True) if do_final else None
    dump_t = {}

    def dump_fm(nm, fm):
        if nm not in dumps:
            return
        w = fm.units[0].shape[1]
        dt_ = fm.units[0].dtype
        dump_t[nm] = dp("dump_" + nm, [len(fm.units), P, w], dt_, isOutput=True)
        for i, u in enumerate(fm.units):
            nc.sync.dma_start(dump_t[nm][i], u[:])

    with tile.TileContext(nc) as tc:
        with ExitStack() as ctx:
            ctx.enter_context(nc.allow_low_precision(
                "bf16/f16/fp8 matmul operands by design; accumulation is f32"))
            pu = ctx.enter_context(tc.tile_pool(name="pu", bufs=u_bufs))
            pw = ctx.enter_context(tc.tile_pool(name="pw", bufs=6))
            pwl = ctx.enter_context(tc.tile_pool(name="pwl", bufs=8))
            pwr = ctx.enter_context(tc.tile_pool(name="pwr", bufs=4))
            pb = ctx.enter_context(tc.tile_pool(name="pb", bufs=10))
            pbv = ctx.enter_context(tc.tile_pool(name="pbv", bufs=1))
            pr = ctx.enter_context(tc.tile_pool(name="pr", bufs=6))
            pst = ctx.enter_context(tc.tile_pool(name="pst", bufs=8))
            pex = ctx.enter_context(tc.tile_pool(name="pex", bufs=8))
            pc = ctx.enter_context(tc.tile_pool(name="pc", bufs=1))
            ps = ctx.enter_context(tc.tile_pool(name="ps", bufs=8, space="PSUM"))

            # ---- constants ----
            ident = pc.tile([P, P], f32, tag="c_id")
            make_identity(nc, ident[:])
            ones_f = pc.tile([P, 1], f32, tag="c_of")
            nc.vector.memset(ones_f[:], 1.0)
            ones_col = pc.tile([P, 1], bf16, tag="c_oc")
            nc.vector.tensor_copy(ones_col[:], ones_f[:])
            ones_rf = pc.tile([1, P], f32, tag="c_orf")
            nc.vector.memset(ones_rf[:], 1.0)
            ones_row = pc.tile([1, P], bf16, tag="c_or")
            nc.vector.tensor_copy(ones_row[:], ones_rf[:])
            lnw_rep = pc.tile([P, D], f32, tag="c_lnw")
            nc.sync.dma_start(lnw_rep[:], ln0w[:].rearrange("(o d) -> o d", o=1)
                              .to_broadcast((P, D)))
            eps_col = pc.tile([P, 1], f32, tag="c_eps")
            nc.vector.memset(eps_col[:], LN_EPS)

            def psum(shape=(P, 512), dtype=f32):
                return ps.tile(list(shape), dtype, tag="ps", name=_nm("ps"))

            # ================= embed =================
            posib = FM(pu, 4, 1024, f32)  # s-chunk major; posi + ln0_b
            for scj in range(4):
                pt = posib.sl(scj)
                nc.sync.dma_start(pt, posib_d[scj * P:(scj + 1) * P, :])

            enc = FM(pu, NDC, 1024, bf16)
            for b in range(BL):
                for sc in range(4):
                    h = pw.tile([P, D], f32, tag="we", bufs=4)
                    nc.sync.dma_start(h[:], xb[sc * P:(sc + 1) * P, b, :])
                    kcol = pst.tile([P, 1], f32, tag="st")
                    nc.sync.dma_start(
                        kcol[:], rnd[b, sc * P:(sc + 1) * P]
                        .rearrange("(p o) -> p o", o=1))
                    km = pst.tile([P, 1], f32, tag="st")
                    nc.vector.tensor_scalar(out=km[:], in0=kcol[:],
                                            scalar1=MASK_RATE, scalar2=None,
                                            op0=OP.is_gt)
                    nc.vector.tensor_scalar(out=h[:], in0=h[:], scalar1=km[:],
                                            scalar2=None, op0=OP.mult)
                    stats = pst.tile([P, 2, 6], f32, tag="st6")
                    hr = h[:].rearrange("p (g f) -> p g f", g=2)
                    for g in range(2):
                        nc.vector.bn_stats(out=stats[:, g, :], in_=hr[:, g, :])
                    mv = pst.tile([P, 2], f32, tag="st")
                    nc.vector.bn_aggr(out=mv[:], in_=stats[:])
                    sd = pst.tile([P, 1], f32, tag="st")
                    nc.scalar.activation(out=sd[:], in_=mv[:, 1:2], func=AF.Sqrt,
                                         bias=eps_col[:])
                    rs = pst.tile([P, 1], f32, tag="st")
                    nc.vector.reciprocal(out=rs[:], in_=sd[:])
                    t1 = pw.tile([P, D], f32, tag="we", bufs=4)
                    nc.vector.scalar_tensor_tensor(
                        out=t1[:], in0=h[:], scalar=mv[:, 0:1],
                        in1=rs[:].to_broadcast((P, D)),
                        op0=OP.subtract, op1=OP.mult)
                    nc.vector.tensor_tensor(out=t1[:], in0=t1[:], in1=lnw_rep[:],
                                            op=OP.mult)
                    nc.vector.tensor_tensor(out=t1[:], in0=t1[:],
                                            in1=posib.sl(sc), op=OP.add)
                    for j in range(NDC):
                        pt = psum((P, P))
                        nc.tensor.transpose(pt[:, :], t1[:, j * P:(j + 1) * P],
                                            ident[:])
                        nc.vector.tensor_copy(
                            enc.sl(j, b * S + sc * P, P), pt[:, :])

            dump_fm("enc0", enc)

            # ================= helpers =================
            def load_cols(src, l, n):
                t = pb.tile([P, n], f32, tag="b")
                nc.sync.dma_start(t[:], src[l])
                return t

            def ln_fm2(blks, w_t, b_t, dst):
                """Batched LN over the feature (partition-chunk) dim.

                blks: list of (X, c0_in, c0_out); all with n=S columns.
                dst[c0_out:c0_out+S] = LN(X[:, c0_in:c0_in+S]) * w + b
                w_t/b_t: per-feature column layout [P, NDC].
                """
                nb = len(blks)
                mu_t = pr.tile([nb, S], f32, tag="r", name=_nm("mu"))
                var_t = pr.tile([nb, S], f32, tag="r", name=_nm("va"))
                for bi, (X, c0_in, _) in enumerate(blks):
                    mu_ps = psum((1, S))
                    sq_ps = psum((1, S))
                    for dc in range(NDC):
                        xs = X.sl(dc, c0_in, S)
                        nc.tensor.matmul(mu_ps[:, :], ones_col[:], xs,
                                         start=(dc == 0), stop=(dc == NDC - 1))
                        sq = pw.tile([P, S], bf16, tag="w")
                        nc.scalar.activation(out=sq[:], in_=xs, func=AF.Square)
                        nc.tensor.matmul(sq_ps[:, :], ones_col[:], sq[:],
                                         start=(dc == 0), stop=(dc == NDC - 1))
                    if bi == 0:
                        mu_w, var_w = mu_t[0:1, :], var_t[0:1, :]
                    else:
                        mu_tmp = pr.tile([1, S], f32, tag="r", name=_nm("mt"))
                        var_tmp = pr.tile([1, S], f32, tag="r", name=_nm("vt"))
                        mu_w, var_w = mu_tmp[:], var_tmp[:]
                    nc.vector.tensor_scalar(
                        out=mu_w, in0=mu_ps[:, :],
                        scalar1=1.0 / D, scalar2=None, op0=OP.mult)
                    mu2 = pr.tile([1, S], f32, tag="r", name=_nm("m2"))
                    nc.vector.tensor_tensor(
                        out=mu2[:], in0=mu_w, in1=mu_w, op=OP.mult)
                    nc.vector.scalar_tensor_tensor(
                        out=var_w, in0=sq_ps[:, :],
                        scalar=1.0 / D, in1=mu2[:],
                        op0=OP.mult, op1=OP.subtract)
                    if bi > 0:
                        nc.sync.dma_start(mu_t[bi:bi + 1, :], mu_w)
                        nc.sync.dma_start(var_t[bi:bi + 1, :], var_w)
                sd_t = pr.tile([nb, S], f32, tag="r", name=_nm("sd"))
                nc.scalar.activation(out=sd_t[:, :], in_=var_t[:, :],
                                     func=AF.Sqrt, bias=eps_col[0:nb, :])
                g_r = pr.tile([nb, S], bf16, tag="r", name=_nm("gr"))
                nc.vector.reciprocal(out=g_r[:, :], in_=sd_t[:, :])
                c_r = pr.tile([nb, S], bf16, tag="r", name=_nm("cr"))
                nc.vector.tensor_tensor(out=c_r[:, :], in0=mu_t[:, :],
                                        in1=g_r[:, :], op=OP.mult)
                for bi, (X, c0_in, c0_out) in enumerate(blks):
                    if bi == 0:
                        g0, c0 = g_r[0:1, :], c_r[0:1, :]
                    else:
                        g0 = pst.tile([1, S], bf16, tag="dn", bufs=4,
                                      name=_nm("g0"))
                        nc.sync.dma_start(g0[:], g_r[bi:bi + 1, :])
                        c0 = pst.tile([1, S], bf16, tag="dn", bufs=4,
                                      name=_nm("c0"))
                        nc.sync.dma_start(c0[:], c_r[bi:bi + 1, :])
                        g0, c0 = g0[:], c0[:]
                    g_ps = psum((P, S))
                    nc.tensor.matmul(g_ps[:, :], ones_row[:], g0,
                                     start=True, stop=True)
                    c_ps = psum((P, S))
                    nc.tensor.matmul(c_ps[:, :], ones_row[:], c0,
                                     start=True, stop=True)
                    for dc in range(NDC):
                        t1 = pw.tile([P, S], bf16, tag="w")
                        nc.vector.tensor_tensor(
                            out=t1[:], in0=X.sl(dc, c0_in, S), in1=g_ps[:, :],
                            op=OP.mult)
                        t2 = pw.tile([P, S], bf16, tag="w")
                        nc.vector.tensor_tensor(
                            out=t2[:], in0=t1[:], in1=c_ps[:, :],
                            op=OP.subtract)
                        nc.scalar.activation(
                            out=dst.sl(dc, c0_out, S), in_=t2[:],
                            func=AF.Identity,
                            scale=w_t[:, dc:dc + 1], bias=b_t[:, dc:dc + 1])

            # ================= layers =================
            for l in range(n_layers):
                bqt = load_cols(bqc, l, NDC)
                bkt = load_cols(bkc, l, NDC)
                bfct = load_cols(bfcc, l, NDC)
                b1t = load_cols(b1c, l, NHC)
                b2t = load_cols(b2c, l, NDC)
                l1wt = load_cols(l1wc, l, NDC)
                l1bt = load_cols(l1bc, l, NDC)
                l2wt = load_cols(l2wc, l, NDC)
                l2bt = load_cols(l2bc, l, NDC)
                bvrep = pbv.tile([P, D], f32, tag="bv")
                nc.sync.dma_start(bvrep[:], bv[l].rearrange("(o d) -> o d", o=1)
                                  .to_broadcast((P, D)))

                # ---- q/k projections, both batches, weights once ----
                qk = {}
                for (nm, W, bt) in (("q", WqT, bqt), ("k", WkT, bkt)):
                    for b in range(BL):
                        qk[(nm, b)] = FM(pu, NDC, S, bf16)
                    for mg in range(NDC // 2):
                        pps = [[psum(), psum()] for _ in range(2)]
                        for k in range(NDC):
                            wt = pwl.tile([P, 256], bf16, tag="wl")
                            nc.sync.dma_start(
                                wt[:], W[l, k * P:(k + 1) * P,
                                         mg * 256:(mg + 1) * 256])
                            for j in range(2):
                                for b in range(BL):
                                    nc.tensor.matmul(
                                        pps[j][b][:, :],
                                        wt[:, j * P:(j + 1) * P],
                                        enc.sl(k, b * S, S),
                                        start=(k == 0), stop=(k == NDC - 1))
                        for j in range(2):
                            m = mg * 2 + j
                            for b in range(BL):
                                nc.scalar.activation(
                                    out=qk[(nm, b)].sl(m), in_=pps[j][b][:, :],
                                    func=AF.Identity, bias=bt[:, m:m + 1])

                if l == 0:
                    dump_fm("q0", qk[("q", 0)])
                    dump_fm("k0", qk[("k", 0)])

                # ---- v projection: token-major, 65 cols/head (65th = ones)
                def vproj(b):
                    vT = FM(pu, 4, H * 65, bf16)
                    for tcc in range(4):
                        u = vT.sl(tcc, 0, H * 65)
                        v3 = u.rearrange("p (h f) -> p h f", f=65)
                        nc.vector.memset(v3[:, :, 64:65], 1.0)
                    for n in range(2):
                        pps = [psum() for _ in range(4)]
                        for k in range(NDC):
                            wt = pwr.tile([P, 512], bf16, tag="wr")
                            nc.sync.dma_start(
                                wt[:], WvT[l, k * P:(k + 1) * P,
                                           n * 512:(n + 1) * 512])
                            for tcc in range(4):
                                nc.tensor.matmul(
                                    pps[tcc][:, :],
                                    enc.sl(k, b * S + tcc * P, P), wt[:],
                                    start=(k == 0), stop=(k == NDC - 1))
                        for tcc in range(4):
                            u = vT.sl(tcc, n * 8 * 65, 8 * 65)
                            dst3 = u.rearrange("p (h f) -> p h f", f=65)
                            nc.vector.tensor_tensor(
                                out=dst3[:, :, 0:64],
                                in0=pps[tcc][:, :]
                                .rearrange("p (h f) -> p h f", f=64),
                                in1=bvrep[:, n * 512:(n + 1) * 512]
                                .rearrange("p (h f) -> p h f", f=64),
                                op=OP.add)
                    return vT

                # ---- attention phase A: scores, exp, AV (+den via ones col)
                def attnA(b, vT):
                    qb, kb = qk[("q", b)], qk[("k", b)]
                    att = FM(pu, NDC, S, bf16)
                    denb = pst.tile([H, S], f32, tag="db", name=_nm("db"),
                                    bufs=2)
                    for h in range(H):
                        dc = h // 2
                        po = (h % 2) * DK
                        exps = []
                        for kc in range(4):
                            sc_ps = psum()
                            nc.tensor.matmul(
                                sc_ps[:, :],
                                kb.sl(dc, kc * P, P, p0=po, np_=DK),
                                qb.sl(dc, 0, S, p0=po, np_=DK),
                                start=True, stop=True)
                            ex = pex.tile([P, S], bf16, tag="ex")
                            nc.scalar.activation(out=ex[:], in_=sc_ps[:, :],
                                                 func=AF.Exp, scale=0.125)
                            exps.append(ex)
                        av_ps = psum((65, S))
                        for kc in range(4):
                            nc.tensor.matmul(
                                av_ps[:, :], vT.sl(kc, h * 65, 65), exps[kc][:],
                                start=(kc == 0), stop=(kc == 3))
                        dtmp = pst.tile([1, S], f32, tag="dn2", bufs=4,
                                        name=_nm("dt"))
                        nc.vector.tensor_copy(dtmp[:], av_ps[64:65, :])
                        nc.sync.dma_start(denb[h:h + 1, :], dtmp[:])
                        nc.vector.tensor_copy(
                            att.sl(dc, 0, S, p0=po, np_=DK), av_ps[0:64, :])
                    return att, denb

                def attn_recip(denb):
                    denr = pr.tile([H, S], bf16, tag="dr", name=_nm("dr"),
                                   bufs=2)
                    nc.vector.reciprocal(out=denr[:, :], in_=denb[:, :])
                    return denr

                # ---- attention phase C: normalize by 1/den
                def attnC(att, denr):
                    for h in range(H):
                        dc = h // 2
                        po = (h % 2) * DK
                        den0 = pst.tile([1, S], bf16, tag="dn", bufs=4,
                                        name=_nm("dn"))
                        nc.sync.dma_start(den0[:], denr[h:h + 1, :])
                        rep_ps = psum((DK, S))
                        nc.tensor.matmul(rep_ps[:, :], ones_row[:, 0:DK],
                                         den0[:], start=True, stop=True)
                        asl = att.sl(dc, 0, S, p0=po, np_=DK)
                        nc.vector.tensor_tensor(out=asl, in0=asl,
                                                in1=rep_ps[:, :], op=OP.mult)

                # ---- fc + bias + residual for batch b ----
                def fc(b, att, C):
                    for mg in range(NDC // 2):
                        pp = [psum(), psum()]
                        for k in range(NDC):
                            wt = pwl.tile([P, 256], bf16, tag="wl")
                            nc.sync.dma_start(
                                wt[:], WfcT[l, k * P:(k + 1) * P,
                                            mg * 256:(mg + 1) * 256])
                            for j in range(2):
                                nc.tensor.matmul(
                                    pp[j][:, :], wt[:, j * P:(j + 1) * P],
                                    att.sl(k), start=(k == 0),
                                    stop=(k == NDC - 1))
                        for j in range(2):
                            m = mg * 2 + j
                            nc.vector.scalar_tensor_tensor(
                                out=C.sl(m), in0=pp[j][:, :],
                                scalar=bfct[:, m:m + 1],
                                in1=enc.sl(m, b * S, S),
                                op0=OP.add, op1=OP.add)

                vT0 = vproj(0)
                att0, den0 = attnA(0, vT0)
                vT1 = vproj(1)
                denr0 = attn_recip(den0)
                attnC(att0, denr0)
                att1, den1 = attnA(1, vT1)
                C = [FM(pu, NDC, S, bf16), FM(pu, NDC, S, bf16)]
                fc(0, att0, C[0])
                denr1 = attn_recip(den1)
                attnC(att1, denr1)
                fc(1, att1, C[1])
                if l == 0:
                    dump_fm("att0", att0)
                    dump_fm("c0", C[0])

                # ---- LN1 -> Dm ----
                Dm = FM(pu, NDC, 1024, bf16)
                ln_fm2([(C[0], 0, 0), (C[1], 0, S)], l1wt, l1bt, Dm)

                # ---- FFN (both halves, weights once) ----
                hid = FM(pu, NHC, 1024, f8)
                for mg in range(NHC // 2):
                    pps = [[psum(), psum()] for _ in range(2)]
                    for k in range(NDC):
                        wt = pwl.tile([P, 256], bf16, tag="wl")
                        nc.sync.dma_start(
                            wt[:], W1T[l, k * P:(k + 1) * P,
                                       mg * 256:(mg + 1) * 256])
                        for j in range(2):
                            for th in range(2):
                                nc.tensor.matmul(
                                    pps[j][th][:, :], wt[:, j * P:(j + 1) * P],
                                    Dm.sl(k, th * S, S),
                                    start=(k == 0), stop=(k == NDC - 1))
                    for j in range(2):
                        m = mg * 2 + j
                        for th in range(2):
                            nc.scalar.activation(
                                out=hid.sl(m, th * S, S), in_=pps[j][th][:, :],
                                func=AF.Relu, bias=b1t[:, m:m + 1])
                E = FM(pu, NDC, 1024, bf16)
                for mg in range(NDC // 2):
                    pps = [[psum(), psum()] for _ in range(2)]
                    for k in range(NHC):
                        wt = pwl.tile([P, 256], f8, tag="wl")
                        nc.sync.dma_start(
                            wt[:], W2T[l, k * P:(k + 1) * P,
                                       mg * 256:(mg + 1) * 256])
                        for j in range(2):
                            for th in range(2):
                                nc.tensor.matmul(
                                    pps[j][th][:, :], wt[:, j * P:(j + 1) * P],
                                    hid.sl(k, th * S, S),
                                    start=(k == 0), stop=(k == NHC - 1))
                    for j in range(2):
                        m = mg * 2 + j
                        for th in range(2):
                            w2o = pw.tile([P, S], bf16, tag="w")
                            nc.scalar.activation(
                                out=w2o[:], in_=pps[j][th][:, :],
                                func=AF.Identity, scale=1.0 / W2_SCALE,
                                bias=b2t[:, m:m + 1])
                            nc.vector.tensor_tensor(
                                out=E.sl(m, th * S, S), in0=w2o[:],
                                in1=Dm.sl(m, th * S, S), op=OP.add)

                # ---- LN2 -> next enc ----
                F = FM(pu, NDC, 1024, bf16)
                ln_fm2([(E, 0, 0), (E, S, S)], l2wt, l2bt, F)
                enc = F
                dump_fm(f"enc_l{l}", enc)

            # ================= final projection + log_softmax =================
            if do_final:
                for tg in range(2):
                    lgu = [pu.tile([P, 4096], f16, tag="u", name=_nm("lg"))
                           for _ in range(10)]

                    def lgsl(tcc, n, ncols):
                        fi = tcc * 10240 + n * 512
                        return lgu[fi // 4096][:, fi % 4096: fi % 4096 + ncols]

                    zaccs = [pst.tile([P, NOC], f32, tag="z", name=_nm("za"))
                             for _ in range(4)]
                    for n in range(NOC):
                        ncols = 512 if n < NOC - 1 else DOUT - (NOC - 1) * 512
                        borep = pb.tile([P, 512], bf16, tag="bon", bufs=4)
                        nc.sync.dma_start(
                            borep[:, :ncols],
                            bo[n * 512:n * 512 + ncols]
                            .rearrange("(o d) -> o d", o=1)
                            .to_broadcast((P, ncols)))
                        pps = [psum() for _ in range(4)]
                        for k in range(NDC):
                            wt = pwr.tile([P, 512], bf16, tag="wr")
                            nc.sync.dma_start(
                                wt[:, :ncols],
                                WoT[k * P:(k + 1) * P, n * 512:n * 512 + ncols])
                            for tcc in range(4):
                                nc.tensor.matmul(
                                    pps[tcc][:, :ncols],
                                    enc.sl(k, tg * S + tcc * P, P),
                                    wt[:, :ncols],
                                    start=(k == 0), stop=(k == NDC - 1))
                        for tcc in range(4):
                            lsl = lgsl(tcc, n, ncols)
                            nc.vector.tensor_tensor(
                                out=lsl, in0=pps[tcc][:, :ncols],
                                in1=borep[:, :ncols], op=OP.add)
                            exs = pw.tile([P, 512], f16, tag="w")
                            nc.scalar.activation(
                                out=exs[:, :ncols], in_=lsl, func=AF.Exp,
                                accum_out=zaccs[tcc][:, n:n + 1])
                    for tcc in range(4):
                        z = pst.tile([P, 1], f32, tag="st")
                        nc.vector.reduce_sum(z[:], zaccs[tcc][:],
                                             axis=mybir.AxisListType.X)
                        lz = pst.tile([P, 1], f32, tag="st")
                        nc.scalar.activation(out=lz[:], in_=z[:], func=AF.Ln)
                        for n in range(NOC):
                            ncols = 512 if n < NOC - 1 else DOUT - (NOC - 1) * 512
                            nc.vector.tensor_scalar(
                                out=lgsl(tcc, n, ncols), in0=lgsl(tcc, n, ncols),
                                scalar1=lz[:], scalar2=None, op0=OP.subtract)
                        # batched output DMA: unit-aligned pieces
                        s0 = tcc * P
                        fi0 = tcc * 10240
                        off = 0
                        while off < DOUT:
                            fi = fi0 + off
                            u, ucol = fi // 4096, fi % 4096
                            ln_ = min(4096 - ucol, DOUT - off)
                            nc.sync.dma_start(
                                out[s0:s0 + P, tg, off:off + ln_],
                                lgu[u][:, ucol:ucol + ln_])
                            off += ln_
    nc.finalize()
    return nc


# ======================= host-side input prep =======================
def make_in_map(inp, core):
    """Build the per-core input dict from the full-problem input dict."""
    import ml_dtypes
    bf = ml_dtypes.bfloat16
    f8n = ml_dtypes.float8_e4m3
    f = np.float32
    c = np.ascontiguousarray
    b0 = core * BL
    m = {
        "xb": c(np.asarray(inp["x"], f)[:, b0:b0 + BL, :]),
        "rnd": c(np.asarray(inp["rnd"], f)[b0:b0 + BL, :]),
        "posib": c(np.asarray(inp["posi"], f)
                   + np.asarray(inp["ln0_b"], f)[None, :]),
        "ln0_w": c(np.asarray(inp["ln0_w"], f)),
        "bv": c(np.asarray(inp["bv"], f)),
        "bo": c(np.asarray(inp["bo"], f).astype(bf)),
    }
    tr = lambda a: c(np.asarray(a, f).transpose(0, 2, 1).astype(bf))
    m["WqT"] = tr(inp["Wq"])
    m["WkT"] = tr(inp["Wk"])
    m["WvT"] = tr(inp["Wv"])
    m["WfcT"] = tr(inp["Wfc"])
    m["W1T"] = tr(inp["W1"])
    m["W2T"] = c((np.asarray(inp["W2"], f).transpose(0, 2, 1)
                  * W2_SCALE).astype(f8n))
    m["WoT"] = c(np.asarray(inp["Wo"], f).T.astype(bf))
    cols = lambda a, nch: c(np.asarray(a, f).reshape(L, nch, P).transpose(0, 2, 1))
    m["bqc"] = cols(inp["bq"], NDC)
    m["bkc"] = cols(inp["bk"], NDC)
    m["bfcc"] = cols(inp["bfc"], NDC)
    m["b1c"] = cols(inp["b1"], NHC)
    m["b2c"] = cols(inp["b2"], NDC)
    m["l1wc"] = cols(inp["ln1_w"], NDC)
    m["l1bc"] = cols(inp["ln1_b"], NDC)
    m["l2wc"] = cols(inp["ln2_w"], NDC)
    m["l2bc"] = cols(inp["ln2_b"], NDC)
    return m


def fm_to_np(arr, nch, ncols, dtype_bytes=4):
    """[n_units, 128, unit_cols] -> [nch*128, ncols]."""
    n_units = arr.shape[0]
    uw = arr.shape[2]
    cpu = uw // ncols
    out = np.zeros((nch * P, ncols), arr.dtype)
    for dc in range(nch):
        u = arr[dc // cpu]
        base = (dc % cpu) * ncols
        out[dc * P:(dc + 1) * P, :] = u[:, base:base + ncols]
    return out


# ======================= entry point =======================
_NC_CACHE = {}


def _get_nc(n_cores=8):
    if n_cores not in _NC_CACHE:
        _NC_CACHE[n_cores] = build(n_layers=L, do_final=True, dumps=(),
                                   n_cores=n_cores)
    return _NC_CACHE[n_cores]


def kernel(**inputs):
    """Full-input, full-output entry point. Shards batch across 8 cores."""
    from concourse.bass_utils import run_bass_kernel_spmd
    n_cores = 8
    nc = _get_nc(n_cores)
    inp = {k: np.asarray(v) for k, v in inputs.items()}
    in_maps = [make_in_map(inp, c) for c in range(n_cores)]
    res = run_bass_kernel_spmd(nc, in_maps, list(range(n_cores)))
    outs = [np.asarray(res.results[c]["out"], np.float32) for c in range(n_cores)]
    return np.concatenate(outs, axis=1)


# revision 64
# speedup vs baseline: 1.8221x; 1.0768x over previous
"""Transformer kernel builder for TRN2 (Bass/Tile), data-parallel over batch.

Per-core: 2 batch elements (T=1024 tokens), full weights.
Feature-major activations [D, T]; bf16 matmuls; fp8 FFN hidden + W2.
"""
import numpy as np
from contextlib import ExitStack

import concourse.bass as bass
import concourse.bacc as bacc
import concourse.tile as tile
from concourse import mybir
from concourse.masks import make_identity

P = 128
S = 512
BL = 2            # local batches per core
T = S * BL        # 1024 tokens per core
D = 1024
H = 16
DK = 64
DHID = 4096
DOUT = 10000
L = 4
LN_EPS = 1e-5
MASK_RATE = 0.15
NDC = D // P      # 8 d-chunks
NHC = DHID // P   # 32 hid chunks
NOC = 20          # dout chunks of 512 (last 272)
W2_SCALE = 64.0   # host scales W2 by this; descaled in the bias activation

f32 = mybir.dt.float32
f16 = mybir.dt.float16
bf16 = mybir.dt.bfloat16
f8 = mybir.dt.float8e4
AF = mybir.ActivationFunctionType
OP = mybir.AluOpType

UW = 2048         # unit width in fp32 elements (8 KiB slots)


_name_ctr = [0]


def _nm(prefix):
    _name_ctr[0] += 1
    return f"{prefix}{_name_ctr[0]}"


def _dtw(dtype):
    return 2 if dtype in (f16, bf16) else (4 if dtype == f8 else 1)


class FM:
    """Chunked buffer: nch chunks of [128, ncols], packed into 8 KiB units."""

    def __init__(self, pool, nch, ncols, dtype):
        self.nch, self.ncols = nch, ncols
        uw = UW * _dtw(dtype)
        self.cpu = max(1, uw // ncols)
        n_units = (nch + self.cpu - 1) // self.cpu
        self.units = [pool.tile([P, self.cpu * ncols], dtype, tag="u",
                                name=_nm("fm"))
                      for _ in range(n_units)]

    def sl(self, dc, c0=0, n=None, p0=0, np_=P):
        n = self.ncols - c0 if n is None else n
        u = self.units[dc // self.cpu]
        base = (dc % self.cpu) * self.ncols
        return u[p0:p0 + np_, base + c0: base + c0 + n]

    def sl2(self, kp, c0=0, n=None):
        """[P, 2, n] AP pairing chunks (2kp, 2kp+1) for DoubleRow matmuls."""
        n = self.ncols - c0 if n is None else n
        k0 = 2 * kp
        assert k0 // self.cpu == (k0 + 1) // self.cpu
        u = self.units[k0 // self.cpu]
        base = (k0 % self.cpu) * self.ncols
        pair = u[0:P, base: base + 2 * self.ncols]
        return pair.rearrange("p (two c) -> p two c", two=2)[:, :, c0:c0 + n]


def build(n_layers=L, do_final=True, dumps=(), n_cores=8, u_bufs=14):
    nc = bacc.Bacc("TRN2", target_bir_lowering=False, debug=False,
                   num_devices=n_cores)
    dp = nc.declare_dram_parameter
    xb = dp("xb", [S, BL, D], f32, isOutput=False)
    rnd = dp("rnd", [BL, S], f32, isOutput=False)
    posib_d = dp("posib", [S, D], f32, isOutput=False)  # posi + ln0_b (host)
    ln0w = dp("ln0_w", [D], f32, isOutput=False)
    WqT = dp("WqT", [L, D, D], bf16, isOutput=False)
    WkT = dp("WkT", [L, D, D], bf16, isOutput=False)
    WvT = dp("WvT", [L, D, D], bf16, isOutput=False)
    WfcT = dp("WfcT", [L, D, D], bf16, isOutput=False)
    W1T = dp("W1T", [L, D, DHID], f8, isOutput=False)
    W2T = dp("W2T", [L, DHID, D], f8, isOutput=False)
    bqc = dp("bqc", [L, P, NDC], f32, isOutput=False)
    bkc = dp("bkc", [L, P, NDC], f32, isOutput=False)
    bfcc = dp("bfcc", [L, P, NDC], f32, isOutput=False)
    b1c = dp("b1c", [L, P, NHC], f32, isOutput=False)
    b2c = dp("b2c", [L, P, NDC], f32, isOutput=False)
    l1wc = dp("l1wc", [L, P, NDC], f32, isOutput=False)
    l1bc = dp("l1bc", [L, P, NDC], f32, isOutput=False)
    l2wc = dp("l2wc", [L, P, NDC], f32, isOutput=False)
    l2bc = dp("l2bc", [L, P, NDC], f32, isOutput=False)
    bv = dp("bv", [L, D], f32, isOutput=False)
    WoT = dp("WoT", [D, DOUT], f8, isOutput=False)
    bo = dp("bo", [DOUT], bf16, isOutput=False)
    out = dp("out", [S, BL, DOUT], f16, isOutput=True) if do_final else None
    dump_t = {}

    def dump_fm(nm, fm):
        if nm not in dumps:
            return
        w = fm.units[0].shape[1]
        dt_ = fm.units[0].dtype
        dump_t[nm] = dp("dump_" + nm, [len(fm.units), P, w], dt_, isOutput=True)
        for i, u in enumerate(fm.units):
            nc.sync.dma_start(dump_t[nm][i], u[:])

    with tile.TileContext(nc) as tc:
        with ExitStack() as ctx:
            ctx.enter_context(nc.allow_low_precision(
                "bf16/f16/fp8 matmul operands by design; accumulation is f32"))
            pu = ctx.enter_context(tc.tile_pool(name="pu", bufs=u_bufs))
            pw = ctx.enter_context(tc.tile_pool(name="pw", bufs=6))
            pwl = ctx.enter_context(tc.tile_pool(name="pwl", bufs=8))
            pwr = ctx.enter_context(tc.tile_pool(name="pwr", bufs=4))
            pb = ctx.enter_context(tc.tile_pool(name="pb", bufs=10))
            pbv = ctx.enter_context(tc.tile_pool(name="pbv", bufs=1))
            pr = ctx.enter_context(tc.tile_pool(name="pr", bufs=6))
            pst = ctx.enter_context(tc.tile_pool(name="pst", bufs=8))
            pex = ctx.enter_context(tc.tile_pool(name="pex", bufs=8))
            pc = ctx.enter_context(tc.tile_pool(name="pc", bufs=1))
            ps = ctx.enter_context(tc.tile_pool(name="ps", bufs=8, space="PSUM"))

            # ---- constants ----
            ident = pc.tile([P, P], f32, tag="c_id")
            make_identity(nc, ident[:])
            ones_f = pc.tile([P, 1], f32, tag="c_of")
            nc.vector.memset(ones_f[:], 1.0)
            ones_col = pc.tile([P, 1], bf16, tag="c_oc")
            nc.vector.tensor_copy(ones_col[:], ones_f[:])
            ones_rf = pc.tile([1, P], f32, tag="c_orf")
            nc.vector.memset(ones_rf[:], 1.0)
            ones_row = pc.tile([1, P], bf16, tag="c_or")
            nc.vector.tensor_copy(ones_row[:], ones_rf[:])
            lnw_rep = pc.tile([P, D], f32, tag="c_lnw")
            nc.sync.dma_start(lnw_rep[:], ln0w[:].rearrange("(o d) -> o d", o=1)
                              .to_broadcast((P, D)))
            eps_col = pc.tile([P, 1], f32, tag="c_eps")
            nc.vector.memset(eps_col[:], LN_EPS)

            def psum(shape=(P, 512), dtype=f32):
                return ps.tile(list(shape), dtype, tag="ps", name=_nm("ps"))

            # ================= embed =================
            posib = FM(pu, 4, 1024, f32)  # s-chunk major; posi + ln0_b
            for scj in range(4):
                pt = posib.sl(scj)
                nc.sync.dma_start(pt, posib_d[scj * P:(scj + 1) * P, :])

            enc = FM(pu, NDC, 1024, bf16)
            for b in range(BL):
                for sc in range(4):
                    h = pw.tile([P, D], f32, tag="we", bufs=4)
                    nc.sync.dma_start(h[:], xb[sc * P:(sc + 1) * P, b, :])
                    kcol = pst.tile([P, 1], f32, tag="st")
                    nc.sync.dma_start(
                        kcol[:], rnd[b, sc * P:(sc + 1) * P]
                        .rearrange("(p o) -> p o", o=1))
                    km = pst.tile([P, 1], f32, tag="st")
                    nc.vector.tensor_scalar(out=km[:], in0=kcol[:],
                                            scalar1=MASK_RATE, scalar2=None,
                                            op0=OP.is_gt)
                    nc.vector.tensor_scalar(out=h[:], in0=h[:], scalar1=km[:],
                                            scalar2=None, op0=OP.mult)
                    stats = pst.tile([P, 2, 6], f32, tag="st6")
                    hr = h[:].rearrange("p (g f) -> p g f", g=2)
                    for g in range(2):
                        nc.vector.bn_stats(out=stats[:, g, :], in_=hr[:, g, :])
                    mv = pst.tile([P, 2], f32, tag="st")
                    nc.vector.bn_aggr(out=mv[:], in_=stats[:])
                    sd = pst.tile([P, 1], f32, tag="st")
                    nc.scalar.activation(out=sd[:], in_=mv[:, 1:2], func=AF.Sqrt,
                                         bias=eps_col[:])
                    rs = pst.tile([P, 1], f32, tag="st")
                    nc.vector.reciprocal(out=rs[:], in_=sd[:])
                    t1 = pw.tile([P, D], f32, tag="we", bufs=4)
                    nc.vector.scalar_tensor_tensor(
                        out=t1[:], in0=h[:], scalar=mv[:, 0:1],
                        in1=rs[:].to_broadcast((P, D)),
                        op0=OP.subtract, op1=OP.mult)
                    nc.vector.tensor_tensor(out=t1[:], in0=t1[:], in1=lnw_rep[:],
                                            op=OP.mult)
                    nc.vector.tensor_tensor(out=t1[:], in0=t1[:],
                                            in1=posib.sl(sc), op=OP.add)
                    for j in range(NDC):
                        pt = psum((P, P))
                        nc.tensor.transpose(pt[:, :], t1[:, j * P:(j + 1) * P],
                                            ident[:])
                        nc.vector.tensor_copy(
                            enc.sl(j, b * S + sc * P, P), pt[:, :])

            dump_fm("enc0", enc)

            # ================= helpers =================
            def load_cols(src, l, n):
                t = pb.tile([P, n], f32, tag="b")
                nc.sync.dma_start(t[:], src[l])
                return t

            def ln_fm2(blks, w_t, b_t, dst):
                """Batched LN over the feature (partition-chunk) dim.

                blks: list of (X, c0_in, c0_out); all with n=S columns.
                dst[c0_out:c0_out+S] = LN(X[:, c0_in:c0_in+S]) * w + b
                w_t/b_t: per-feature column layout [P, NDC].
                """
                nb = len(blks)
                mu_t = pr.tile([nb, S], f32, tag="r", name=_nm("mu"))
                var_t = pr.tile([nb, S], f32, tag="r", name=_nm("va"))
                for bi, (X, c0_in, _) in enumerate(blks):
                    mu_ps = psum((1, S))
                    sq_ps = psum((1, S))
                    for dc in range(NDC):
                        xs = X.sl(dc, c0_in, S)
                        nc.tensor.matmul(mu_ps[:, :], ones_col[:], xs,
                                         start=(dc == 0), stop=(dc == NDC - 1))
                        sq = pw.tile([P, S], bf16, tag="w")
                        nc.scalar.activation(out=sq[:], in_=xs, func=AF.Square)
                        nc.tensor.matmul(sq_ps[:, :], ones_col[:], sq[:],
                                         start=(dc == 0), stop=(dc == NDC - 1))
                    if bi == 0:
                        mu_w, var_w = mu_t[0:1, :], var_t[0:1, :]
                    else:
                        mu_tmp = pr.tile([1, S], f32, tag="r", name=_nm("mt"))
                        var_tmp = pr.tile([1, S], f32, tag="r", name=_nm("vt"))
                        mu_w, var_w = mu_tmp[:], var_tmp[:]
                    nc.vector.tensor_scalar(
                        out=mu_w, in0=mu_ps[:, :],
                        scalar1=1.0 / D, scalar2=None, op0=OP.mult)
                    mu2 = pr.tile([1, S], f32, tag="r", name=_nm("m2"))
                    nc.vector.tensor_tensor(
                        out=mu2[:], in0=mu_w, in1=mu_w, op=OP.mult)
                    nc.vector.scalar_tensor_tensor(
                        out=var_w, in0=sq_ps[:, :],
                        scalar=1.0 / D, in1=mu2[:],
                        op0=OP.mult, op1=OP.subtract)
                    if bi > 0:
                        nc.sync.dma_start(mu_t[bi:bi + 1, :], mu_w)
                        nc.sync.dma_start(var_t[bi:bi + 1, :], var_w)
                sd_t = pr.tile([nb, S], f32, tag="r", name=_nm("sd"))
                nc.scalar.activation(out=sd_t[:, :], in_=var_t[:, :],
                                     func=AF.Sqrt, bias=eps_col[0:nb, :])
                g_r = pr.tile([nb, S], bf16, tag="r", name=_nm("gr"))
                nc.vector.reciprocal(out=g_r[:, :], in_=sd_t[:, :])
                c_r = pr.tile([nb, S], bf16, tag="r", name=_nm("cr"))
                nc.vector.tensor_tensor(out=c_r[:, :], in0=mu_t[:, :],
                                        in1=g_r[:, :], op=OP.mult)
                for bi, (X, c0_in, c0_out) in enumerate(blks):
                    if bi == 0:
                        g0, c0 = g_r[0:1, :], c_r[0:1, :]
                    else:
                        g0 = pst.tile([1, S], bf16, tag="dn", bufs=4,
                                      name=_nm("g0"))
                        nc.sync.dma_start(g0[:], g_r[bi:bi + 1, :])
                        c0 = pst.tile([1, S], bf16, tag="dn", bufs=4,
                                      name=_nm("c0"))
                        nc.sync.dma_start(c0[:], c_r[bi:bi + 1, :])
                        g0, c0 = g0[:], c0[:]
                    g_ps = psum((P, S))
                    nc.tensor.matmul(g_ps[:, :], ones_row[:], g0,
                                     start=True, stop=True)
                    c_ps = psum((P, S))
                    nc.tensor.matmul(c_ps[:, :], ones_row[:], c0,
                                     start=True, stop=True)
                    for dc in range(NDC):
                        t1 = pw.tile([P, S], bf16, tag="w")
                        nc.vector.tensor_tensor(
                            out=t1[:], in0=X.sl(dc, c0_in, S), in1=g_ps[:, :],
                            op=OP.mult)
                        t2 = pw.tile([P, S], bf16, tag="w")
                        nc.vector.tensor_tensor(
                            out=t2[:], in0=t1[:], in1=c_ps[:, :],
                            op=OP.subtract)
                        nc.scalar.activation(
                            out=dst.sl(dc, c0_out, S), in_=t2[:],
                            func=AF.Identity,
                            scale=w_t[:, dc:dc + 1], bias=b_t[:, dc:dc + 1])

            # ================= layers =================
            for l in range(n_layers):
                bqt = load_cols(bqc, l, NDC)
                bkt = load_cols(bkc, l, NDC)
                bfct = load_cols(bfcc, l, NDC)
                b1t = load_cols(b1c, l, NHC)
                b2t = load_cols(b2c, l, NDC)
                l1wt = load_cols(l1wc, l, NDC)
                l1bt = load_cols(l1bc, l, NDC)
                l2wt = load_cols(l2wc, l, NDC)
                l2bt = load_cols(l2bc, l, NDC)
                bvrep = pbv.tile([P, D], f32, tag="bv")
                nc.sync.dma_start(bvrep[:], bv[l].rearrange("(o d) -> o d", o=1)
                                  .to_broadcast((P, D)))

                # ---- q/k projections, both batches, weights once ----
                qk = {}
                for (nm, W, bt) in (("q", WqT, bqt), ("k", WkT, bkt)):
                    for b in range(BL):
                        qk[(nm, b)] = FM(pu, NDC, S, bf16)
                    for mg in range(NDC // 2):
                        pps = [[psum(), psum()] for _ in range(2)]
                        for k in range(NDC):
                            wt = pwl.tile([P, 256], bf16, tag="wl")
                            nc.sync.dma_start(
                                wt[:], W[l, k * P:(k + 1) * P,
                                         mg * 256:(mg + 1) * 256])
                            for j in range(2):
                                for b in range(BL):
                                    nc.tensor.matmul(
                                        pps[j][b][:, :],
                                        wt[:, j * P:(j + 1) * P],
                                        enc.sl(k, b * S, S),
                                        start=(k == 0), stop=(k == NDC - 1))
                        for j in range(2):
                            m = mg * 2 + j
                            for b in range(BL):
                                nc.scalar.activation(
                                    out=qk[(nm, b)].sl(m), in_=pps[j][b][:, :],
                                    func=AF.Identity, bias=bt[:, m:m + 1])

                if l == 0:
                    dump_fm("q0", qk[("q", 0)])
                    dump_fm("k0", qk[("k", 0)])

                # ---- v projection: token-major, 65 cols/head (65th = ones)
                def vproj(b):
                    vT = FM(pu, 4, H * 65, bf16)
                    for tcc in range(4):
                        u = vT.sl(tcc, 0, H * 65)
                        v3 = u.rearrange("p (h f) -> p h f", f=65)
                        nc.vector.memset(v3[:, :, 64:65], 1.0)
                    for n in range(2):
                        pps = [psum() for _ in range(4)]
                        for k in range(NDC):
                            wt = pwr.tile([P, 512], bf16, tag="wr")
                            nc.sync.dma_start(
                                wt[:], WvT[l, k * P:(k + 1) * P,
                                           n * 512:(n + 1) * 512])
                            for tcc in range(4):
                                nc.tensor.matmul(
                                    pps[tcc][:, :],
                                    enc.sl(k, b * S + tcc * P, P), wt[:],
                                    start=(k == 0), stop=(k == NDC - 1))
                        for tcc in range(4):
                            u = vT.sl(tcc, n * 8 * 65, 8 * 65)
                            dst3 = u.rearrange("p (h f) -> p h f", f=65)
                            nc.vector.tensor_tensor(
                                out=dst3[:, :, 0:64],
                                in0=pps[tcc][:, :]
                                .rearrange("p (h f) -> p h f", f=64),
                                in1=bvrep[:, n * 512:(n + 1) * 512]
                                .rearrange("p (h f) -> p h f", f=64),
                                op=OP.add)
                    return vT

                # ---- attention phase A: scores, exp, AV (+den via ones col)
                def attnA(b, vT):
                    qb, kb = qk[("q", b)], qk[("k", b)]
                    att = FM(pu, NDC, S, bf16)
                    denb = pst.tile([H, S], f32, tag="db", name=_nm("db"),
                                    bufs=2)
                    for h in range(H):
                        dc = h // 2
                        po = (h % 2) * DK
                        exps = []
                        for kc in range(4):
                            sc_ps = psum()
                            nc.tensor.matmul(
                                sc_ps[:, :],
                                kb.sl(dc, kc * P, P, p0=po, np_=DK),
                                qb.sl(dc, 0, S, p0=po, np_=DK),
                                start=True, stop=True)
                            ex = pex.tile([P, S], bf16, tag="ex")
                            nc.scalar.activation(out=ex[:], in_=sc_ps[:, :],
                                                 func=AF.Exp, scale=0.125)
                            exps.append(ex)
                        av_ps = psum((65, S))
                        for kc in range(4):
                            nc.tensor.matmul(
                                av_ps[:, :], vT.sl(kc, h * 65, 65), exps[kc][:],
                                start=(kc == 0), stop=(kc == 3))
                        dtmp = pst.tile([1, S], f32, tag="dn2", bufs=4,
                                        name=_nm("dt"))
                        nc.vector.tensor_copy(dtmp[:], av_ps[64:65, :])
                        nc.sync.dma_start(denb[h:h + 1, :], dtmp[:])
                        nc.vector.tensor_copy(
                            att.sl(dc, 0, S, p0=po, np_=DK), av_ps[0:64, :])
                    return att, denb

                def attn_recip(denb):
                    denr = pr.tile([H, S], bf16, tag="dr", name=_nm("dr"),
                                   bufs=2)
                    nc.vector.reciprocal(out=denr[:, :], in_=denb[:, :])
                    return denr

                # ---- attention phase C: normalize by 1/den
                def attnC(att, denr):
                    for dc in range(NDC):
                        rep_ps = psum((P, S))
                        for i in range(2):
                            h = 2 * dc + i
                            den0 = pst.tile([1, S], bf16, tag="dn", bufs=4,
                                            name=_nm("dn"))
                            nc.sync.dma_start(den0[:], denr[h:h + 1, :])
                            nc.tensor.matmul(rep_ps[i * DK:(i + 1) * DK, :],
                                             ones_row[:, 0:DK], den0[:],
                                             start=True, stop=True,
                                             tile_position=(0, i * DK),
                                             skip_group_check=True)
                        asl = att.sl(dc, 0, S)
                        nc.vector.tensor_tensor(out=asl, in0=asl,
                                                in1=rep_ps[:, :], op=OP.mult)

                # ---- fc + bias + residual for batch b ----
                def fc(b, att, C):
                    for mg in range(NDC // 2):
                        pp = [psum(), psum()]
                        for k in range(NDC):
                            wt = pwl.tile([P, 256], bf16, tag="wl")
                            nc.sync.dma_start(
                                wt[:], WfcT[l, k * P:(k + 1) * P,
                                            mg * 256:(mg + 1) * 256])
                            for j in range(2):
                                nc.tensor.matmul(
                                    pp[j][:, :], wt[:, j * P:(j + 1) * P],
                                    att.sl(k), start=(k == 0),
                                    stop=(k == NDC - 1))
                        for j in range(2):
                            m = mg * 2 + j
                            nc.vector.scalar_tensor_tensor(
                                out=C.sl(m), in0=pp[j][:, :],
                                scalar=bfct[:, m:m + 1],
                                in1=enc.sl(m, b * S, S),
                                op0=OP.add, op1=OP.add)

                vT0 = vproj(0)
                att0, den0 = attnA(0, vT0)
                vT1 = vproj(1)
                denr0 = attn_recip(den0)
                attnC(att0, denr0)
                att1, den1 = attnA(1, vT1)
                C = [FM(pu, NDC, S, bf16), FM(pu, NDC, S, bf16)]
                fc(0, att0, C[0])
                denr1 = attn_recip(den1)
                attnC(att1, denr1)
                fc(1, att1, C[1])
                if l == 0:
                    dump_fm("att0", att0)
                    dump_fm("c0", C[0])

                # ---- LN1 -> Dm ----
                Dm = FM(pu, NDC, 1024, bf16)
                ln_fm2([(C[0], 0, 0), (C[1], 0, S)], l1wt, l1bt, Dm)

                # ---- FFN (both halves, weights once) ----
                Dm8 = FM(pu, NDC, 1024, f8)
                for dc in range(NDC):
                    nc.vector.tensor_copy(Dm8.sl(dc), Dm.sl(dc))
                hid = FM(pu, NHC, 1024, f8)
                for mg in range(NHC // 2):
                    pps = [[psum(), psum()] for _ in range(2)]
                    for kp in range(NDC // 2):
                        wt = pwl.tile([P, 512], f8, tag="wl")
                        wt3 = wt[:].rearrange("p (two c) -> p two c", two=2)
                        nc.sync.dma_start(
                            wt3, W1T[l, 2 * kp * P:(2 * kp + 2) * P,
                                     mg * 256:(mg + 1) * 256]
                            .rearrange("(two p) c -> p two c", two=2))
                        for j in range(2):
                            for th in range(2):
                                nc.tensor.matmul(
                                    pps[j][th][:, :],
                                    wt3[:, :, j * P:(j + 1) * P],
                                    Dm8.sl2(kp, th * S, S),
                                    start=(kp == 0), stop=(kp == NDC // 2 - 1),
                                    perf_mode=mybir.MatmulPerfMode.DoubleRow)
                    for j in range(2):
                        m = mg * 2 + j
                        for th in range(2):
                            nc.scalar.activation(
                                out=hid.sl(m, th * S, S), in_=pps[j][th][:, :],
                                func=AF.Relu, scale=1.0 / W2_SCALE,
                                bias=b1t[:, m:m + 1])
                E = FM(pu, NDC, 1024, bf16)
                for mg in range(NDC // 2):
                    pps = [[psum(), psum()] for _ in range(2)]
                    for kp in range(NHC // 2):
                        wt = pwl.tile([P, 512], f8, tag="wl")
                        wt3 = wt[:].rearrange("p (two c) -> p two c", two=2)
                        nc.sync.dma_start(
                            wt3, W2T[l, 2 * kp * P:(2 * kp + 2) * P,
                                     mg * 256:(mg + 1) * 256]
                            .rearrange("(two p) c -> p two c", two=2))
                        for j in range(2):
                            for th in range(2):
                                nc.tensor.matmul(
                                    pps[j][th][:, :],
                                    wt3[:, :, j * P:(j + 1) * P],
                                    hid.sl2(kp, th * S, S),
                                    start=(kp == 0), stop=(kp == NHC // 2 - 1),
                                    perf_mode=mybir.MatmulPerfMode.DoubleRow)
                    for j in range(2):
                        m = mg * 2 + j
                        for th in range(2):
                            w2o = pw.tile([P, S], bf16, tag="w")
                            nc.scalar.activation(
                                out=w2o[:], in_=pps[j][th][:, :],
                                func=AF.Identity, scale=1.0 / W2_SCALE,
                                bias=b2t[:, m:m + 1])
                            nc.vector.tensor_tensor(
                                out=E.sl(m, th * S, S), in0=w2o[:],
                                in1=Dm.sl(m, th * S, S), op=OP.add)

                # ---- LN2 -> next enc (fp8 for the final Wo GEMM) ----
                F = FM(pu, NDC, 1024, f8 if l == n_layers - 1 else bf16)
                ln_fm2([(E, 0, 0), (E, S, S)], l2wt, l2bt, F)
                enc = F
                dump_fm(f"enc_l{l}", enc)

            # ================= final projection + log_softmax =================
            if do_final:
                for tg in range(2):
                    lgu = [pu.tile([P, 4096], f16, tag="u", name=_nm("lg"))
                           for _ in range(10)]

                    def lgsl(tcc, n, ncols):
                        fi = tcc * 10240 + n * 512
                        return lgu[fi // 4096][:, fi % 4096: fi % 4096 + ncols]

                    zaccs = [pst.tile([P, NOC], f32, tag="z", name=_nm("za"))
                             for _ in range(4)]
                    for n in range(NOC):
                        ncols = 512 if n < NOC - 1 else DOUT - (NOC - 1) * 512
                        borep = pb.tile([P, 512], bf16, tag="bon", bufs=4)
                        nc.sync.dma_start(
                            borep[:, :ncols],
                            bo[n * 512:n * 512 + ncols]
                            .rearrange("(o d) -> o d", o=1)
                            .to_broadcast((P, ncols)))
                        pps = [psum() for _ in range(4)]
                        for kp in range(NDC // 2):
                            wt = pwr.tile([P, 1024], f8, tag="wr")
                            wt3 = wt[:, :2 * ncols].rearrange(
                                "p (two c) -> p two c", two=2)
                            nc.sync.dma_start(
                                wt3, WoT[2 * kp * P:(2 * kp + 2) * P,
                                         n * 512:n * 512 + ncols]
                                .rearrange("(two p) c -> p two c", two=2))
                            for tcc in range(4):
                                nc.tensor.matmul(
                                    pps[tcc][:, :ncols],
                                    enc.sl2(kp, tg * S + tcc * P, P),
                                    wt3[:, :, :],
                                    start=(kp == 0), stop=(kp == NDC // 2 - 1),
                                    perf_mode=mybir.MatmulPerfMode.DoubleRow)
                        for tcc in range(4):
                            lsl = lgsl(tcc, n, ncols)
                            nc.vector.scalar_tensor_tensor(
                                out=lsl, in0=pps[tcc][:, :ncols],
                                scalar=1.0 / W2_SCALE, in1=borep[:, :ncols],
                                op0=OP.mult, op1=OP.add)
                            exs = pw.tile([P, 512], f16, tag="w")
                            nc.scalar.activation(
                                out=exs[:, :ncols], in_=lsl, func=AF.Exp,
                                accum_out=zaccs[tcc][:, n:n + 1])
                    for tcc in range(4):
                        z = pst.tile([P, 1], f32, tag="st")
                        nc.vector.reduce_sum(z[:], zaccs[tcc][:],
                                             axis=mybir.AxisListType.X)
                        lz = pst.tile([P, 1], f32, tag="st")
                        nc.scalar.activation(out=lz[:], in_=z[:], func=AF.Ln)
                        for n in range(NOC):
                            ncols = 512 if n < NOC - 1 else DOUT - (NOC - 1) * 512
                            nc.vector.tensor_scalar(
                                out=lgsl(tcc, n, ncols), in0=lgsl(tcc, n, ncols),
                                scalar1=lz[:], scalar2=None, op0=OP.subtract)
                        # batched output DMA: unit-aligned pieces
                        s0 = tcc * P
                        fi0 = tcc * 10240
                        off = 0
                        while off < DOUT:
                            fi = fi0 + off
                            u, ucol = fi // 4096, fi % 4096
                            ln_ = min(4096 - ucol, DOUT - off)
                            nc.sync.dma_start(
                                out[s0:s0 + P, tg, off:off + ln_],
                                lgu[u][:, ucol:ucol + ln_])
                            off += ln_
    nc.finalize()
    return nc


# ======================= host-side input prep =======================
def make_in_map(inp, core):
    """Build the per-core input dict from the full-problem input dict."""
    import ml_dtypes
    bf = ml_dtypes.bfloat16
    f8n = ml_dtypes.float8_e4m3
    f = np.float32
    c = np.ascontiguousarray
    b0 = core * BL
    m = {
        "xb": c(np.asarray(inp["x"], f)[:, b0:b0 + BL, :]),
        "rnd": c(np.asarray(inp["rnd"], f)[b0:b0 + BL, :]),
        "posib": c(np.asarray(inp["posi"], f)
                   + np.asarray(inp["ln0_b"], f)[None, :]),
        "ln0_w": c(np.asarray(inp["ln0_w"], f)),
        "bv": c(np.asarray(inp["bv"], f)),
        "bo": c(np.asarray(inp["bo"], f).astype(bf)),
    }
    tr = lambda a: c(np.asarray(a, f).transpose(0, 2, 1).astype(bf))
    m["WqT"] = tr(inp["Wq"])
    m["WkT"] = tr(inp["Wk"])
    m["WvT"] = tr(inp["Wv"])
    m["WfcT"] = tr(inp["Wfc"])
    m["W1T"] = c((np.asarray(inp["W1"], f).transpose(0, 2, 1)
                  * W2_SCALE).astype(f8n))
    m["W2T"] = c((np.asarray(inp["W2"], f).transpose(0, 2, 1)
                  * W2_SCALE).astype(f8n))
    m["WoT"] = c((np.asarray(inp["Wo"], f).T * W2_SCALE).astype(f8n))
    cols = lambda a, nch: c(np.asarray(a, f).reshape(L, nch, P).transpose(0, 2, 1))
    m["bqc"] = cols(inp["bq"], NDC)
    m["bkc"] = cols(inp["bk"], NDC)
    m["bfcc"] = cols(inp["bfc"], NDC)
    m["b1c"] = cols(inp["b1"], NHC)
    m["b2c"] = cols(inp["b2"], NDC)
    m["l1wc"] = cols(inp["ln1_w"], NDC)
    m["l1bc"] = cols(inp["ln1_b"], NDC)
    m["l2wc"] = cols(inp["ln2_w"], NDC)
    m["l2bc"] = cols(inp["ln2_b"], NDC)
    return m


def fm_to_np(arr, nch, ncols, dtype_bytes=4):
    """[n_units, 128, unit_cols] -> [nch*128, ncols]."""
    n_units = arr.shape[0]
    uw = arr.shape[2]
    cpu = uw // ncols
    out = np.zeros((nch * P, ncols), arr.dtype)
    for dc in range(nch):
        u = arr[dc // cpu]
        base = (dc % cpu) * ncols
        out[dc * P:(dc + 1) * P, :] = u[:, base:base + ncols]
    return out


# ======================= entry point =======================
_NC_CACHE = {}


def _get_nc(n_cores=8):
    if n_cores not in _NC_CACHE:
        _NC_CACHE[n_cores] = build(n_layers=L, do_final=True, dumps=(),
                                   n_cores=n_cores)
    return _NC_CACHE[n_cores]


def kernel(**inputs):
    """Full-input, full-output entry point. Shards batch across 8 cores."""
    from concourse.bass_utils import run_bass_kernel_spmd
    n_cores = 8
    nc = _get_nc(n_cores)
    inp = {k: np.asarray(v) for k, v in inputs.items()}
    in_maps = [make_in_map(inp, c) for c in range(n_cores)]
    res = run_bass_kernel_spmd(nc, in_maps, list(range(n_cores)))
    outs = [np.asarray(res.results[c]["out"], np.float32) for c in range(n_cores)]
    return np.concatenate(outs, axis=1)


# revision 81
# speedup vs baseline: 2.0233x; 1.1104x over previous
"""Transformer kernel builder for TRN2 (Bass/Tile), data-parallel over batch.

Per-core: 2 batch elements (T=1024 tokens), full weights.
Feature-major activations [D, T]; bf16 matmuls; fp8 FFN hidden + W2.
"""
import numpy as np
from contextlib import ExitStack

import concourse.bass as bass
import concourse.bacc as bacc
import concourse.tile as tile
from concourse import mybir
from concourse.masks import make_identity

P = 128
S = 512
BL = 2            # local batches per core
T = S * BL        # 1024 tokens per core
D = 1024
H = 16
DK = 64
DHID = 4096
DOUT = 10000
L = 4
LN_EPS = 1e-5
MASK_RATE = 0.15
NDC = D // P      # 8 d-chunks
NHC = DHID // P   # 32 hid chunks
NOC = 20          # dout chunks of 512 (last 272)
W2_SCALE = 64.0   # host scales W2 by this; descaled in the bias activation

f32 = mybir.dt.float32
f16 = mybir.dt.float16
bf16 = mybir.dt.bfloat16
f8 = mybir.dt.float8e4
AF = mybir.ActivationFunctionType
OP = mybir.AluOpType

UW = 2048         # unit width in fp32 elements (8 KiB slots)


_name_ctr = [0]


def _nm(prefix):
    _name_ctr[0] += 1
    return f"{prefix}{_name_ctr[0]}"


def _dtw(dtype):
    return 2 if dtype in (f16, bf16) else (4 if dtype == f8 else 1)


class FM:
    """Chunked buffer: nch chunks of [128, ncols], packed into 8 KiB units."""

    def __init__(self, pool, nch, ncols, dtype):
        self.nch, self.ncols = nch, ncols
        uw = UW * _dtw(dtype)
        self.cpu = max(1, uw // ncols)
        n_units = (nch + self.cpu - 1) // self.cpu
        self.units = [pool.tile([P, self.cpu * ncols], dtype, tag="u",
                                name=_nm("fm"))
                      for _ in range(n_units)]

    def sl(self, dc, c0=0, n=None, p0=0, np_=P):
        n = self.ncols - c0 if n is None else n
        u = self.units[dc // self.cpu]
        base = (dc % self.cpu) * self.ncols
        return u[p0:p0 + np_, base + c0: base + c0 + n]

    def sl2(self, kp, c0=0, n=None):
        """[P, 2, n] AP pairing chunks (2kp, 2kp+1) for DoubleRow matmuls."""
        n = self.ncols - c0 if n is None else n
        k0 = 2 * kp
        assert k0 // self.cpu == (k0 + 1) // self.cpu
        u = self.units[k0 // self.cpu]
        base = (k0 % self.cpu) * self.ncols
        pair = u[0:P, base: base + 2 * self.ncols]
        return pair.rearrange("p (two c) -> p two c", two=2)[:, :, c0:c0 + n]


def build(n_layers=L, do_final=True, dumps=(), n_cores=8, u_bufs=14):
    nc = bacc.Bacc("TRN2", target_bir_lowering=False, debug=False,
                   num_devices=n_cores)
    dp = nc.declare_dram_parameter
    xb = dp("xb", [S, BL, D], f32, isOutput=False)
    rnd = dp("rnd", [BL, S], f32, isOutput=False)
    posib_d = dp("posib", [S, D], f32, isOutput=False)  # posi + ln0_b (host)
    ln0w = dp("ln0_w", [D], f32, isOutput=False)
    WqT = dp("WqT", [L, D, D], bf16, isOutput=False)
    WkT = dp("WkT", [L, D, D], bf16, isOutput=False)
    WvT = dp("WvT", [L, D, D], bf16, isOutput=False)
    WfcT = dp("WfcT", [L, D, D], bf16, isOutput=False)
    W1T = dp("W1T", [L, D, DHID], f8, isOutput=False)
    W2T = dp("W2T", [L, DHID, D], f8, isOutput=False)
    bqc = dp("bqc", [L, P, NDC], f32, isOutput=False)
    bkc = dp("bkc", [L, P, NDC], f32, isOutput=False)
    bfcc = dp("bfcc", [L, P, NDC], f32, isOutput=False)
    b1c = dp("b1c", [L, P, NHC], f32, isOutput=False)
    b2c = dp("b2c", [L, P, NDC], f32, isOutput=False)
    l1wc = dp("l1wc", [L, P, NDC], f32, isOutput=False)
    l1bc = dp("l1bc", [L, P, NDC], f32, isOutput=False)
    l2wc = dp("l2wc", [L, P, NDC], f32, isOutput=False)
    l2bc = dp("l2bc", [L, P, NDC], f32, isOutput=False)
    bv = dp("bv", [L, D], f32, isOutput=False)
    WoT = dp("WoT", [D, DOUT], f8, isOutput=False)
    bo = dp("bo", [DOUT], bf16, isOutput=False)
    out = dp("out", [S, BL, DOUT], f16, isOutput=True) if do_final else None
    dump_t = {}

    def dump_fm(nm, fm):
        if nm not in dumps:
            return
        w = fm.units[0].shape[1]
        dt_ = fm.units[0].dtype
        dump_t[nm] = dp("dump_" + nm, [len(fm.units), P, w], dt_, isOutput=True)
        for i, u in enumerate(fm.units):
            nc.sync.dma_start(dump_t[nm][i], u[:])

    with tile.TileContext(nc) as tc:
        with ExitStack() as ctx:
            ctx.enter_context(nc.allow_low_precision(
                "bf16/f16/fp8 matmul operands by design; accumulation is f32"))
            pu = ctx.enter_context(tc.tile_pool(name="pu", bufs=u_bufs))
            pw = ctx.enter_context(tc.tile_pool(name="pw", bufs=6))
            pwl = ctx.enter_context(tc.tile_pool(name="pwl", bufs=8))
            pwr = ctx.enter_context(tc.tile_pool(name="pwr", bufs=4))
            pb = ctx.enter_context(tc.tile_pool(name="pb", bufs=10))
            pbv = ctx.enter_context(tc.tile_pool(name="pbv", bufs=1))
            pr = ctx.enter_context(tc.tile_pool(name="pr", bufs=6))
            pst = ctx.enter_context(tc.tile_pool(name="pst", bufs=8))
            pex = ctx.enter_context(tc.tile_pool(name="pex", bufs=6))
            pc = ctx.enter_context(tc.tile_pool(name="pc", bufs=1))
            ps = ctx.enter_context(tc.tile_pool(name="ps", bufs=2, space="PSUM"))

            # ---- constants ----
            ident = pc.tile([P, P], f32, tag="c_id")
            make_identity(nc, ident[:])
            ones_f = pc.tile([P, 1], f32, tag="c_of")
            nc.vector.memset(ones_f[:], 1.0)
            ones_col = pc.tile([P, 1], bf16, tag="c_oc")
            nc.vector.tensor_copy(ones_col[:], ones_f[:])
            ones_rf = pc.tile([1, P], f32, tag="c_orf")
            nc.vector.memset(ones_rf[:], 1.0)
            ones_row = pc.tile([1, P], bf16, tag="c_or")
            nc.vector.tensor_copy(ones_row[:], ones_rf[:])
            lnw_rep = pc.tile([P, D], f32, tag="c_lnw")
            nc.sync.dma_start(lnw_rep[:], ln0w[:].rearrange("(o d) -> o d", o=1)
                              .to_broadcast((P, D)))
            eps_col = pc.tile([P, 1], f32, tag="c_eps")
            nc.vector.memset(eps_col[:], LN_EPS)

            def psum(shape=(P, 512), dtype=f32):
                return ps.tile(list(shape), dtype, tag="ps", name=_nm("ps"),
                               bufs=2)

            def psum2():
                return ps.tile([P, 1024], f32, tag="ps2", name=_nm("p2"),
                               bufs=3)

            # ================= embed =================
            posib = FM(pu, 4, 1024, f32)  # s-chunk major; posi + ln0_b
            for scj in range(4):
                pt = posib.sl(scj)
                nc.sync.dma_start(pt, posib_d[scj * P:(scj + 1) * P, :])

            enc = FM(pu, NDC, 1024, bf16)
            for b in range(BL):
                for sc in range(4):
                    h = pw.tile([P, D], f32, tag="we", bufs=4)
                    nc.sync.dma_start(h[:], xb[sc * P:(sc + 1) * P, b, :])
                    kcol = pst.tile([P, 1], f32, tag="st")
                    nc.sync.dma_start(
                        kcol[:], rnd[b, sc * P:(sc + 1) * P]
                        .rearrange("(p o) -> p o", o=1))
                    km = pst.tile([P, 1], f32, tag="st")
                    nc.vector.tensor_scalar(out=km[:], in0=kcol[:],
                                            scalar1=MASK_RATE, scalar2=None,
                                            op0=OP.is_gt)
                    nc.gpsimd.tensor_scalar(out=h[:], in0=h[:], scalar1=km[:],
                                            scalar2=None, op0=OP.mult)
                    stats = pst.tile([P, 2, 6], f32, tag="st6")
                    hr = h[:].rearrange("p (g f) -> p g f", g=2)
                    for g in range(2):
                        nc.vector.bn_stats(out=stats[:, g, :], in_=hr[:, g, :])
                    mv = pst.tile([P, 2], f32, tag="st")
                    nc.vector.bn_aggr(out=mv[:], in_=stats[:])
                    sd = pst.tile([P, 1], f32, tag="st")
                    nc.scalar.activation(out=sd[:], in_=mv[:, 1:2], func=AF.Sqrt,
                                         bias=eps_col[:])
                    rs = pst.tile([P, 1], f32, tag="st")
                    nc.vector.reciprocal(out=rs[:], in_=sd[:])
                    t1 = pw.tile([P, D], f32, tag="we", bufs=4)
                    nc.vector.scalar_tensor_tensor(
                        out=t1[:], in0=h[:], scalar=mv[:, 0:1],
                        in1=rs[:].to_broadcast((P, D)),
                        op0=OP.subtract, op1=OP.mult)
                    nc.vector.tensor_tensor(out=t1[:], in0=t1[:], in1=lnw_rep[:],
                                            op=OP.mult)
                    nc.gpsimd.tensor_tensor(out=t1[:], in0=t1[:],
                                            in1=posib.sl(sc), op=OP.add)
                    for j in range(NDC):
                        pt = psum((P, P))
                        nc.tensor.transpose(pt[:, :], t1[:, j * P:(j + 1) * P],
                                            ident[:])
                        nc.vector.tensor_copy(
                            enc.sl(j, b * S + sc * P, P), pt[:, :])

            dump_fm("enc0", enc)

            # ================= helpers =================
            def load_cols(src, l, n):
                t = pb.tile([P, n], f32, tag="b")
                nc.sync.dma_start(t[:], src[l])
                return t

            def ln_fm2(blks, w_t, b_t, dst, dst8=None):
                """Batched LN over the feature (partition-chunk) dim.

                blks: list of (X, c0_in); block bi writes dst cols
                [bi*S, (bi+1)*S). dst8: optional fp8 shadow of dst.
                """
                nb = len(blks)
                mu_t = pr.tile([nb, S], f32, tag="r", name=_nm("mu"))
                var_t = pr.tile([nb, S], f32, tag="r", name=_nm("va"))
                for bi, (X, c0_in) in enumerate(blks):
                    st_ps = psum2()
                    mu_ps = st_ps[0:1, 0:S]
                    sq_ps = st_ps[0:1, S:2 * S]
                    for dc in range(NDC):
                        xs = X.sl(dc, c0_in, S)
                        nc.tensor.matmul(mu_ps, ones_col[:], xs,
                                         start=(dc == 0), stop=(dc == NDC - 1))
                        sq = pw.tile([P, S], bf16, tag="w")
                        nc.scalar.activation(out=sq[:], in_=xs, func=AF.Square)
                        nc.tensor.matmul(sq_ps, ones_col[:], sq[:],
                                         start=(dc == 0), stop=(dc == NDC - 1))
                    if bi == 0:
                        mu_w, var_w = mu_t[0:1, :], var_t[0:1, :]
                    else:
                        mu_tmp = pr.tile([1, S], f32, tag="r", name=_nm("mt"))
                        var_tmp = pr.tile([1, S], f32, tag="r", name=_nm("vt"))
                        mu_w, var_w = mu_tmp[:], var_tmp[:]
                    nc.vector.tensor_scalar(
                        out=mu_w, in0=mu_ps,
                        scalar1=1.0 / D, scalar2=None, op0=OP.mult)
                    mu2 = pr.tile([1, S], f32, tag="r", name=_nm("m2"))
                    nc.vector.tensor_tensor(
                        out=mu2[:], in0=mu_w, in1=mu_w, op=OP.mult)
                    nc.vector.scalar_tensor_tensor(
                        out=var_w, in0=sq_ps,
                        scalar=1.0 / D, in1=mu2[:],
                        op0=OP.mult, op1=OP.subtract)
                    if bi > 0:
                        nc.sync.dma_start(mu_t[bi:bi + 1, :], mu_w)
                        nc.sync.dma_start(var_t[bi:bi + 1, :], var_w)
                sd_t = pr.tile([nb, S], f32, tag="r", name=_nm("sd"))
                nc.scalar.activation(out=sd_t[:, :], in_=var_t[:, :],
                                     func=AF.Sqrt, bias=eps_col[0:nb, :])
                g_r = pr.tile([nb, S], bf16, tag="r", name=_nm("gr"))
                nc.vector.reciprocal(out=g_r[:, :], in_=sd_t[:, :])
                c_r = pr.tile([nb, S], bf16, tag="r", name=_nm("cr"))
                nc.vector.tensor_tensor(out=c_r[:, :], in0=mu_t[:, :],
                                        in1=g_r[:, :], op=OP.mult)
                gc = []
                for bi in range(nb):
                    if bi == 0:
                        g0, c0 = g_r[0:1, :], c_r[0:1, :]
                    else:
                        g0 = pst.tile([1, S], bf16, tag="dn", bufs=4,
                                      name=_nm("g0"))
                        nc.sync.dma_start(g0[:], g_r[bi:bi + 1, :])
                        c0 = pst.tile([1, S], bf16, tag="dn", bufs=4,
                                      name=_nm("c0"))
                        nc.sync.dma_start(c0[:], c_r[bi:bi + 1, :])
                        g0, c0 = g0[:], c0[:]
                    gc_ps = psum2()
                    nc.tensor.matmul(gc_ps[:, 0:S], ones_row[:], g0,
                                     start=True, stop=True)
                    nc.tensor.matmul(gc_ps[:, S:2 * S], ones_row[:], c0,
                                     start=True, stop=True)
                    gc.append(gc_ps)
                for dc in range(NDC):
                    t2p = pw.tile([P, 2 * S], bf16, tag="wp", bufs=4)
                    for bi, (X, c0_in) in enumerate(blks):
                        t1 = pw.tile([P, S], bf16, tag="w")
                        nc.vector.tensor_tensor(
                            out=t1[:], in0=X.sl(dc, c0_in, S),
                            in1=gc[bi][:, 0:S], op=OP.mult)
                        nc.vector.tensor_tensor(
                            out=t2p[:, bi * S:(bi + 1) * S], in0=t1[:],
                            in1=gc[bi][:, S:2 * S], op=OP.subtract)
                    nc.scalar.activation(
                        out=dst.sl(dc, 0, 2 * S), in_=t2p[:],
                        func=AF.Identity,
                        scale=w_t[:, dc:dc + 1], bias=b_t[:, dc:dc + 1])
                    if dst8 is not None:
                        nc.gpsimd.tensor_copy(dst8.sl(dc, 0, 2 * S),
                                              dst.sl(dc, 0, 2 * S))

            # ================= layers =================
            for l in range(n_layers):
                bqt = load_cols(bqc, l, NDC)
                bkt = load_cols(bkc, l, NDC)
                bfct = load_cols(bfcc, l, NDC)
                b1t = load_cols(b1c, l, NHC)
                b2t = load_cols(b2c, l, NDC)
                l1wt = load_cols(l1wc, l, NDC)
                l1bt = load_cols(l1bc, l, NDC)
                l2wt = load_cols(l2wc, l, NDC)
                l2bt = load_cols(l2bc, l, NDC)
                bvrep = pbv.tile([P, D], f32, tag="bv")
                nc.sync.dma_start(bvrep[:], bv[l].rearrange("(o d) -> o d", o=1)
                                  .to_broadcast((P, D)))

                # ---- q/k projections, both batches, weights once ----
                qk = {}
                for (nm, W, bt) in (("q", WqT, bqt), ("k", WkT, bkt)):
                    qk[nm] = FM(pu, NDC, T, bf16)
                    for mg in range(NDC // 2):
                        pps = [psum2(), psum2()]
                        for k in range(NDC):
                            wt = pwl.tile([P, 256], bf16, tag="wl")
                            nc.sync.dma_start(
                                wt[:], W[l, k * P:(k + 1) * P,
                                         mg * 256:(mg + 1) * 256])
                            for j in range(2):
                                for b in range(BL):
                                    nc.tensor.matmul(
                                        pps[j][:, b * S:(b + 1) * S],
                                        wt[:, j * P:(j + 1) * P],
                                        enc.sl(k, b * S, S),
                                        start=(k == 0), stop=(k == NDC - 1))
                        for j in range(2):
                            m = mg * 2 + j
                            nc.scalar.activation(
                                out=qk[nm].sl(m), in_=pps[j][:, :],
                                func=AF.Identity, bias=bt[:, m:m + 1])

                if l == 0:
                    dump_fm("q0", qk["q"])
                    dump_fm("k0", qk["k"])

                # ---- v projection: token-major, 65 cols/head (65th = ones)
                def vproj(b):
                    vT = FM(pu, 4, H * 65, bf16)
                    for tcc in range(4):
                        u = vT.sl(tcc, 0, H * 65)
                        v3 = u.rearrange("p (h f) -> p h f", f=65)
                        nc.vector.memset(v3[:, :, 64:65], 1.0)
                    for n in range(2):
                        pps = [psum2(), psum2()]
                        for k in range(NDC):
                            wt = pwr.tile([P, 512], bf16, tag="wr")
                            nc.sync.dma_start(
                                wt[:], WvT[l, k * P:(k + 1) * P,
                                           n * 512:(n + 1) * 512])
                            for tcc in range(4):
                                nc.tensor.matmul(
                                    pps[tcc // 2][:, (tcc % 2) * S:
                                                  (tcc % 2 + 1) * S],
                                    enc.sl(k, b * S + tcc * P, P), wt[:],
                                    start=(k == 0), stop=(k == NDC - 1))
                        for tcc in range(4):
                            u = vT.sl(tcc, n * 8 * 65, 8 * 65)
                            dst3 = u.rearrange("p (h f) -> p h f", f=65)
                            nc.vector.tensor_tensor(
                                out=dst3[:, :, 0:64],
                                in0=pps[tcc // 2][:, (tcc % 2) * S:
                                                  (tcc % 2 + 1) * S]
                                .rearrange("p (h f) -> p h f", f=64),
                                in1=bvrep[:, n * 512:(n + 1) * 512]
                                .rearrange("p (h f) -> p h f", f=64),
                                op=OP.add)
                    return vT

                # ---- attention phase A: scores, exp, AV (+den via ones col)
                def attnA(b, vT):
                    qb, kb = qk["q"], qk["k"]
                    att = FM(pu, NDC, S, bf16)
                    denb = pst.tile([H, S], f32, tag="db", name=_nm("db"),
                                    bufs=2)
                    for h in range(H):
                        dc = h // 2
                        po = (h % 2) * DK
                        exps = []
                        for kcp in range(2):
                            sc2 = psum2()
                            for i in range(2):
                                kc = 2 * kcp + i
                                nc.tensor.matmul(
                                    sc2[:, i * S:(i + 1) * S],
                                    kb.sl(dc, b * S + kc * P, P,
                                          p0=po, np_=DK),
                                    qb.sl(dc, b * S, S, p0=po, np_=DK),
                                    start=True, stop=True)
                            ex = pex.tile([P, 2 * S], bf16, tag="ex")
                            nc.scalar.activation(out=ex[:], in_=sc2[:, :],
                                                 func=AF.Exp, scale=0.125)
                            exps.append(ex)
                        av_ps = psum((65, S))
                        for kc in range(4):
                            nc.tensor.matmul(
                                av_ps[:, :], vT.sl(kc, h * 65, 65),
                                exps[kc // 2][:, (kc % 2) * S:(kc % 2 + 1) * S],
                                start=(kc == 0), stop=(kc == 3))
                        dtmp = pst.tile([1, S], f32, tag="dn2", bufs=4,
                                        name=_nm("dt"))
                        nc.vector.tensor_copy(dtmp[:], av_ps[64:65, :])
                        nc.sync.dma_start(denb[h:h + 1, :], dtmp[:])
                        nc.vector.tensor_copy(
                            att.sl(dc, 0, S, p0=po, np_=DK), av_ps[0:64, :])
                    return att, denb

                def attn_recip(denb):
                    denr = pr.tile([H, S], bf16, tag="dr", name=_nm("dr"),
                                   bufs=2)
                    nc.vector.reciprocal(out=denr[:, :], in_=denb[:, :])
                    return denr

                # ---- attention phase C: normalize by 1/den
                def attnC(att, denr):
                    for dc in range(NDC):
                        rep_ps = psum((P, S))
                        for i in range(2):
                            h = 2 * dc + i
                            den0 = pst.tile([1, S], bf16, tag="dn", bufs=4,
                                            name=_nm("dn"))
                            nc.sync.dma_start(den0[:], denr[h:h + 1, :])
                            nc.tensor.matmul(rep_ps[i * DK:(i + 1) * DK, :],
                                             ones_row[:, 0:DK], den0[:],
                                             start=True, stop=True,
                                             tile_position=(0, i * DK),
                                             skip_group_check=True)
                        asl = att.sl(dc, 0, S)
                        nc.vector.tensor_tensor(out=asl, in0=asl,
                                                in1=rep_ps[:, :], op=OP.mult)

                # ---- fc + bias + residual for batch b ----
                def fc(b, att, C):
                    for mg in range(NDC // 2):
                        pp = psum2()
                        for k in range(NDC):
                            wt = pwl.tile([P, 256], bf16, tag="wl")
                            nc.sync.dma_start(
                                wt[:], WfcT[l, k * P:(k + 1) * P,
                                            mg * 256:(mg + 1) * 256])
                            for j in range(2):
                                nc.tensor.matmul(
                                    pp[:, j * S:(j + 1) * S],
                                    wt[:, j * P:(j + 1) * P],
                                    att.sl(k), start=(k == 0),
                                    stop=(k == NDC - 1))
                        for j in range(2):
                            m = mg * 2 + j
                            nc.vector.scalar_tensor_tensor(
                                out=C.sl(m), in0=pp[:, j * S:(j + 1) * S],
                                scalar=bfct[:, m:m + 1],
                                in1=enc.sl(m, b * S, S),
                                op0=OP.add, op1=OP.add)

                vT0 = vproj(0)
                att0, den0 = attnA(0, vT0)
                vT1 = vproj(1)
                denr0 = attn_recip(den0)
                attnC(att0, denr0)
                att1, den1 = attnA(1, vT1)
                C = [FM(pu, NDC, S, bf16), FM(pu, NDC, S, bf16)]
                fc(0, att0, C[0])
                denr1 = attn_recip(den1)
                attnC(att1, denr1)
                fc(1, att1, C[1])
                if l == 0:
                    dump_fm("att0", att0)
                    dump_fm("c0", C[0])

                # ---- LN1 -> Dm (+ fp8 shadow for W1) ----
                Dm = FM(pu, NDC, 1024, bf16)
                Dm8 = FM(pu, NDC, 1024, f8)
                ln_fm2([(C[0], 0), (C[1], 0)], l1wt, l1bt, Dm, Dm8)

                # ---- FFN (both halves, weights once) ----
                hid = FM(pu, NHC, 1024, f8)
                for mg in range(NHC // 2):
                    pps = [psum2(), psum2()]
                    for kp in range(NDC // 2):
                        wt = pwl.tile([P, 512], f8, tag="wl")
                        wt3 = wt[:].rearrange("p (two c) -> p two c", two=2)
                        nc.sync.dma_start(
                            wt3, W1T[l, 2 * kp * P:(2 * kp + 2) * P,
                                     mg * 256:(mg + 1) * 256]
                            .rearrange("(two p) c -> p two c", two=2))
                        for j in range(2):
                            for th in range(2):
                                nc.tensor.matmul(
                                    pps[j][:, th * S:(th + 1) * S],
                                    wt3[:, :, j * P:(j + 1) * P],
                                    Dm8.sl2(kp, th * S, S),
                                    start=(kp == 0), stop=(kp == NDC // 2 - 1),
                                    perf_mode=mybir.MatmulPerfMode.DoubleRow)
                    for j in range(2):
                        m = mg * 2 + j
                        nc.scalar.activation(
                            out=hid.sl(m, 0, T), in_=pps[j][:, :],
                            func=AF.Relu, scale=1.0 / W2_SCALE,
                            bias=b1t[:, m:m + 1])
                E = FM(pu, NDC, 1024, bf16)
                for mg in range(NDC // 2):
                    pps = [psum2(), psum2()]
                    for kp in range(NHC // 2):
                        wt = pwl.tile([P, 512], f8, tag="wl")
                        wt3 = wt[:].rearrange("p (two c) -> p two c", two=2)
                        nc.sync.dma_start(
                            wt3, W2T[l, 2 * kp * P:(2 * kp + 2) * P,
                                     mg * 256:(mg + 1) * 256]
                            .rearrange("(two p) c -> p two c", two=2))
                        for j in range(2):
                            for th in range(2):
                                nc.tensor.matmul(
                                    pps[j][:, th * S:(th + 1) * S],
                                    wt3[:, :, j * P:(j + 1) * P],
                                    hid.sl2(kp, th * S, S),
                                    start=(kp == 0), stop=(kp == NHC // 2 - 1),
                                    perf_mode=mybir.MatmulPerfMode.DoubleRow)
                    for j in range(2):
                        m = mg * 2 + j
                        w2o = pw.tile([P, T], bf16, tag="wp", bufs=4)
                        nc.scalar.activation(
                            out=w2o[:], in_=pps[j][:, :],
                            func=AF.Identity, scale=1.0 / W2_SCALE,
                            bias=b2t[:, m:m + 1])
                        nc.vector.tensor_tensor(
                            out=E.sl(m, 0, T), in0=w2o[:],
                            in1=Dm.sl(m, 0, T), op=OP.add)

                # ---- LN2 -> next enc (fp8 for the final Wo GEMM) ----
                F = FM(pu, NDC, 1024, f8 if l == n_layers - 1 else bf16)
                ln_fm2([(E, 0), (E, S)], l2wt, l2bt, F)
                enc = F
                dump_fm(f"enc_l{l}", enc)

            # ================= final projection + log_softmax =================
            if do_final:
                for tg in range(2):
                    lgu = [pu.tile([P, 4096], f16, tag="u", name=_nm("lg"))
                           for _ in range(10)]

                    def lgsl(tcc, n, ncols):
                        fi = tcc * 10240 + n * 512
                        return lgu[fi // 4096][:, fi % 4096: fi % 4096 + ncols]

                    zaccs = [pst.tile([P, NOC // 2], f32, tag="z",
                                      name=_nm("za")) for _ in range(4)]
                    for n in range(NOC):
                        ncols = 512 if n < NOC - 1 else DOUT - (NOC - 1) * 512
                        borep = pb.tile([P, 512], bf16, tag="bon", bufs=4)
                        nc.sync.dma_start(
                            borep[:, :ncols],
                            bo[n * 512:n * 512 + ncols]
                            .rearrange("(o d) -> o d", o=1)
                            .to_broadcast((P, ncols)))
                        pps = [psum2(), psum2()]
                        for kp in range(NDC // 2):
                            wt = pwr.tile([P, 1024], f8, tag="wr")
                            wt3 = wt[:, :2 * ncols].rearrange(
                                "p (two c) -> p two c", two=2)
                            nc.sync.dma_start(
                                wt3, WoT[2 * kp * P:(2 * kp + 2) * P,
                                         n * 512:n * 512 + ncols]
                                .rearrange("(two p) c -> p two c", two=2))
                            for tcc in range(4):
                                nc.tensor.matmul(
                                    pps[tcc // 2][:, (tcc % 2) * S:
                                                  (tcc % 2) * S + ncols],
                                    enc.sl2(kp, tg * S + tcc * P, P),
                                    wt3[:, :, :],
                                    start=(kp == 0), stop=(kp == NDC // 2 - 1),
                                    perf_mode=mybir.MatmulPerfMode.DoubleRow)
                        for tcc in range(4):
                            lsl = lgsl(tcc, n, ncols)
                            nc.vector.scalar_tensor_tensor(
                                out=lsl, in0=pps[tcc // 2][:, (tcc % 2) * S:
                                                           (tcc % 2) * S + ncols],
                                scalar=1.0 / W2_SCALE, in1=borep[:, :ncols],
                                op0=OP.mult, op1=OP.add)
                            if n % 2 == 1:
                                exs = pw.tile([P, 2 * S], f16, tag="wp",
                                              bufs=4)
                                nc.scalar.activation(
                                    out=exs[:, :512 + ncols],
                                    in_=lgsl(tcc, n - 1, 512 + ncols),
                                    func=AF.Exp,
                                    accum_out=zaccs[tcc][:, n // 2:n // 2 + 1])
                    for tcc in range(4):
                        z = pst.tile([P, 1], f32, tag="st")
                        nc.vector.reduce_sum(z[:], zaccs[tcc][:],
                                             axis=mybir.AxisListType.X)
                        lz = pst.tile([P, 1], f32, tag="st")
                        nc.scalar.activation(out=lz[:], in_=z[:], func=AF.Ln)
                        for n in range(NOC):
                            ncols = 512 if n < NOC - 1 else DOUT - (NOC - 1) * 512
                            nc.vector.tensor_scalar(
                                out=lgsl(tcc, n, ncols), in0=lgsl(tcc, n, ncols),
                                scalar1=lz[:], scalar2=None, op0=OP.subtract)
                        # batched output DMA: unit-aligned pieces
                        s0 = tcc * P
                        fi0 = tcc * 10240
                        off = 0
                        while off < DOUT:
                            fi = fi0 + off
                            u, ucol = fi // 4096, fi % 4096
                            ln_ = min(4096 - ucol, DOUT - off)
                            nc.sync.dma_start(
                                out[s0:s0 + P, tg, off:off + ln_],
                                lgu[u][:, ucol:ucol + ln_])
                            off += ln_
    nc.finalize()
    return nc


# ======================= host-side input prep =======================
def make_in_map(inp, core):
    """Build the per-core input dict from the full-problem input dict."""
    import ml_dtypes
    bf = ml_dtypes.bfloat16
    f8n = ml_dtypes.float8_e4m3
    f = np.float32
    c = np.ascontiguousarray
    b0 = core * BL
    m = {
        "xb": c(np.asarray(inp["x"], f)[:, b0:b0 + BL, :]),
        "rnd": c(np.asarray(inp["rnd"], f)[b0:b0 + BL, :]),
        "posib": c(np.asarray(inp["posi"], f)
                   + np.asarray(inp["ln0_b"], f)[None, :]),
        "ln0_w": c(np.asarray(inp["ln0_w"], f)),
        "bv": c(np.asarray(inp["bv"], f)),
        "bo": c(np.asarray(inp["bo"], f).astype(bf)),
    }
    tr = lambda a: c(np.asarray(a, f).transpose(0, 2, 1).astype(bf))
    m["WqT"] = tr(inp["Wq"])
    m["WkT"] = tr(inp["Wk"])
    m["WvT"] = tr(inp["Wv"])
    m["WfcT"] = tr(inp["Wfc"])
    m["W1T"] = c((np.asarray(inp["W1"], f).transpose(0, 2, 1)
                  * W2_SCALE).astype(f8n))
    m["W2T"] = c((np.asarray(inp["W2"], f).transpose(0, 2, 1)
                  * W2_SCALE).astype(f8n))
    m["WoT"] = c((np.asarray(inp["Wo"], f).T * W2_SCALE).astype(f8n))
    cols = lambda a, nch: c(np.asarray(a, f).reshape(L, nch, P).transpose(0, 2, 1))
    m["bqc"] = cols(inp["bq"], NDC)
    m["bkc"] = cols(inp["bk"], NDC)
    m["bfcc"] = cols(inp["bfc"], NDC)
    m["b1c"] = cols(inp["b1"], NHC)
    m["b2c"] = cols(inp["b2"], NDC)
    m["l1wc"] = cols(inp["ln1_w"], NDC)
    m["l1bc"] = cols(inp["ln1_b"], NDC)
    m["l2wc"] = cols(inp["ln2_w"], NDC)
    m["l2bc"] = cols(inp["ln2_b"], NDC)
    return m


def fm_to_np(arr, nch, ncols, dtype_bytes=4):
    """[n_units, 128, unit_cols] -> [nch*128, ncols]."""
    n_units = arr.shape[0]
    uw = arr.shape[2]
    cpu = uw // ncols
    out = np.zeros((nch * P, ncols), arr.dtype)
    for dc in range(nch):
        u = arr[dc // cpu]
        base = (dc % cpu) * ncols
        out[dc * P:(dc + 1) * P, :] = u[:, base:base + ncols]
    return out


# ======================= entry point =======================
_NC_CACHE = {}


def _get_nc(n_cores=8):
    if n_cores not in _NC_CACHE:
        _NC_CACHE[n_cores] = build(n_layers=L, do_final=True, dumps=(),
                                   n_cores=n_cores)
    return _NC_CACHE[n_cores]


def kernel(**inputs):
    """Full-input, full-output entry point. Shards batch across 8 cores."""
    from concourse.bass_utils import run_bass_kernel_spmd
    n_cores = 8
    nc = _get_nc(n_cores)
    inp = {k: np.asarray(v) for k, v in inputs.items()}
    in_maps = [make_in_map(inp, c) for c in range(n_cores)]
    res = run_bass_kernel_spmd(nc, in_maps, list(range(n_cores)))
    outs = [np.asarray(res.results[c]["out"], np.float32) for c in range(n_cores)]
    return np.concatenate(outs, axis=1)


# revision 92
# speedup vs baseline: 2.2424x; 1.1083x over previous
"""Transformer kernel builder for TRN2 (Bass/Tile), data-parallel over batch.

Per-core: 2 batch elements (T=1024 tokens), full weights.
Feature-major activations [D, T]; bf16 matmuls; fp8 FFN hidden + W2.
"""
import numpy as np
from contextlib import ExitStack

import concourse.bass as bass
import concourse.bacc as bacc
import concourse.tile as tile
from concourse import mybir
from concourse.masks import make_identity

P = 128
S = 512
BL = 2            # local batches per core
T = S * BL        # 1024 tokens per core
D = 1024
H = 16
DK = 64
DHID = 4096
DOUT = 10000
L = 4
LN_EPS = 1e-5
MASK_RATE = 0.15
NDC = D // P      # 8 d-chunks
NHC = DHID // P   # 32 hid chunks
NOC = 20          # dout chunks of 512 (last 272)
W2_SCALE = 64.0   # host scales W2 by this; descaled in the bias activation

f32 = mybir.dt.float32
f16 = mybir.dt.float16
bf16 = mybir.dt.bfloat16
f8 = mybir.dt.float8e4
AF = mybir.ActivationFunctionType
OP = mybir.AluOpType

UW = 2048         # unit width in fp32 elements (8 KiB slots)


_name_ctr = [0]


def _nm(prefix):
    _name_ctr[0] += 1
    return f"{prefix}{_name_ctr[0]}"


def _dtw(dtype):
    return 2 if dtype in (f16, bf16) else (4 if dtype == f8 else 1)


class FM:
    """Chunked buffer: nch chunks of [128, ncols], packed into 8 KiB units."""

    def __init__(self, pool, nch, ncols, dtype):
        self.nch, self.ncols = nch, ncols
        uw = UW * _dtw(dtype)
        self.cpu = max(1, uw // ncols)
        n_units = (nch + self.cpu - 1) // self.cpu
        self.units = [pool.tile([P, self.cpu * ncols], dtype, tag="u",
                                name=_nm("fm"))
                      for _ in range(n_units)]

    def sl(self, dc, c0=0, n=None, p0=0, np_=P):
        n = self.ncols - c0 if n is None else n
        u = self.units[dc // self.cpu]
        base = (dc % self.cpu) * self.ncols
        return u[p0:p0 + np_, base + c0: base + c0 + n]

    def sl2(self, kp, c0=0, n=None):
        """[P, 2, n] AP pairing chunks (2kp, 2kp+1) for DoubleRow matmuls."""
        n = self.ncols - c0 if n is None else n
        k0 = 2 * kp
        assert k0 // self.cpu == (k0 + 1) // self.cpu
        u = self.units[k0 // self.cpu]
        base = (k0 % self.cpu) * self.ncols
        pair = u[0:P, base: base + 2 * self.ncols]
        return pair.rearrange("p (two c) -> p two c", two=2)[:, :, c0:c0 + n]


def build(n_layers=L, do_final=True, dumps=(), n_cores=8, u_bufs=14):
    nc = bacc.Bacc("TRN2", target_bir_lowering=False, debug=False,
                   num_devices=n_cores)
    dp = nc.declare_dram_parameter
    xb = dp("xb", [S, BL, D], f32, isOutput=False)
    rnd = dp("rnd", [BL, S], f32, isOutput=False)
    posib_d = dp("posib", [S, D], f32, isOutput=False)  # posi + ln0_b (host)
    ln0w = dp("ln0_w", [D], f32, isOutput=False)
    WqT = dp("WqT", [L, D, D], bf16, isOutput=False)
    WkT = dp("WkT", [L, D, D], bf16, isOutput=False)
    WvT = dp("WvT", [L, D, D], bf16, isOutput=False)
    WfcT = dp("WfcT", [L, D, D], bf16, isOutput=False)
    W1T = dp("W1T", [L, D, DHID], f8, isOutput=False)
    W2T = dp("W2T", [L, DHID, D], f8, isOutput=False)
    bqc = dp("bqc", [L, P, NDC], f32, isOutput=False)
    bkc = dp("bkc", [L, P, NDC], f32, isOutput=False)
    bfcc = dp("bfcc", [L, P, NDC], f32, isOutput=False)
    b1c = dp("b1c", [L, P, NHC], f32, isOutput=False)
    b2c = dp("b2c", [L, P, NDC], f32, isOutput=False)
    l1wc = dp("l1wc", [L, P, NDC], f32, isOutput=False)
    l1bc = dp("l1bc", [L, P, NDC], f32, isOutput=False)
    l2wc = dp("l2wc", [L, P, NDC], f32, isOutput=False)
    l2bc = dp("l2bc", [L, P, NDC], f32, isOutput=False)
    bv = dp("bv", [L, D], f32, isOutput=False)
    WoT = dp("WoT", [D, DOUT], f8, isOutput=False)
    bo = dp("bo", [DOUT], bf16, isOutput=False)
    out = dp("out", [S, BL, DOUT], f16, isOutput=True) if do_final else None
    dump_t = {}

    def dump_fm(nm, fm):
        if nm not in dumps:
            return
        w = fm.units[0].shape[1]
        dt_ = fm.units[0].dtype
        dump_t[nm] = dp("dump_" + nm, [len(fm.units), P, w], dt_, isOutput=True)
        for i, u in enumerate(fm.units):
            nc.sync.dma_start(dump_t[nm][i], u[:])

    with tile.TileContext(nc) as tc:
        with ExitStack() as ctx:
            ctx.enter_context(nc.allow_low_precision(
                "bf16/f16/fp8 matmul operands by design; accumulation is f32"))
            pu = ctx.enter_context(tc.tile_pool(name="pu", bufs=u_bufs))
            pw = ctx.enter_context(tc.tile_pool(name="pw", bufs=6))
            pwl = ctx.enter_context(tc.tile_pool(name="pwl", bufs=8))
            pwr = ctx.enter_context(tc.tile_pool(name="pwr", bufs=3))
            pb = ctx.enter_context(tc.tile_pool(name="pb", bufs=10))
            pbv = ctx.enter_context(tc.tile_pool(name="pbv", bufs=1))
            pr = ctx.enter_context(tc.tile_pool(name="pr", bufs=6))
            pst = ctx.enter_context(tc.tile_pool(name="pst", bufs=8))
            pex = ctx.enter_context(tc.tile_pool(name="pex", bufs=6))
            pc = ctx.enter_context(tc.tile_pool(name="pc", bufs=1))
            ps = ctx.enter_context(tc.tile_pool(name="ps", bufs=2, space="PSUM"))

            # ---- constants ----
            ident = pc.tile([P, P], f32, tag="c_id")
            make_identity(nc, ident[:])
            ones_f = pc.tile([P, 1], f32, tag="c_of")
            nc.vector.memset(ones_f[:], 1.0)
            ones_col = pc.tile([P, 1], bf16, tag="c_oc")
            nc.vector.tensor_copy(ones_col[:], ones_f[:])
            ones_rf = pc.tile([1, P], f32, tag="c_orf")
            nc.vector.memset(ones_rf[:], 1.0)
            ones_row = pc.tile([1, P], bf16, tag="c_or")
            nc.vector.tensor_copy(ones_row[:], ones_rf[:])
            lnw_rep = pc.tile([P, D], f32, tag="c_lnw")
            nc.sync.dma_start(lnw_rep[:], ln0w[:].rearrange("(o d) -> o d", o=1)
                              .to_broadcast((P, D)))
            eps_col = pc.tile([P, 1], f32, tag="c_eps")
            nc.vector.memset(eps_col[:], LN_EPS)

            def psum(shape=(P, 512), dtype=f32):
                return ps.tile(list(shape), dtype, tag="ps", name=_nm("ps"),
                               bufs=2)

            def psum2():
                return ps.tile([P, 1024], f32, tag="ps2", name=_nm("p2"),
                               bufs=3)

            # ================= embed =================
            posib = FM(pu, 4, 1024, f32)  # s-chunk major; posi + ln0_b
            for scj in range(4):
                pt = posib.sl(scj)
                nc.sync.dma_start(pt, posib_d[scj * P:(scj + 1) * P, :])

            enc = FM(pu, NDC, 1024, bf16)
            for b in range(BL):
                for sc in range(4):
                    h = pw.tile([P, D], f32, tag="we", bufs=3)
                    nc.sync.dma_start(h[:], xb[sc * P:(sc + 1) * P, b, :])
                    kcol = pst.tile([P, 1], f32, tag="st")
                    nc.sync.dma_start(
                        kcol[:], rnd[b, sc * P:(sc + 1) * P]
                        .rearrange("(p o) -> p o", o=1))
                    km = pst.tile([P, 1], f32, tag="st")
                    nc.vector.tensor_scalar(out=km[:], in0=kcol[:],
                                            scalar1=MASK_RATE, scalar2=None,
                                            op0=OP.is_gt)
                    nc.vector.tensor_scalar(out=h[:], in0=h[:], scalar1=km[:],
                                            scalar2=None, op0=OP.mult)
                    stats = pst.tile([P, 2, 6], f32, tag="st6")
                    hr = h[:].rearrange("p (g f) -> p g f", g=2)
                    for g in range(2):
                        nc.vector.bn_stats(out=stats[:, g, :], in_=hr[:, g, :])
                    mv = pst.tile([P, 2], f32, tag="st")
                    nc.vector.bn_aggr(out=mv[:], in_=stats[:])
                    sd = pst.tile([P, 1], f32, tag="st")
                    nc.scalar.activation(out=sd[:], in_=mv[:, 1:2], func=AF.Sqrt,
                                         bias=eps_col[:])
                    rs = pst.tile([P, 1], f32, tag="st")
                    nc.vector.reciprocal(out=rs[:], in_=sd[:])
                    t1 = pw.tile([P, D], f32, tag="we", bufs=3)
                    nc.vector.scalar_tensor_tensor(
                        out=t1[:], in0=h[:], scalar=mv[:, 0:1],
                        in1=rs[:].to_broadcast((P, D)),
                        op0=OP.subtract, op1=OP.mult)
                    nc.vector.tensor_tensor(out=t1[:], in0=t1[:], in1=lnw_rep[:],
                                            op=OP.mult)
                    nc.gpsimd.tensor_tensor(out=t1[:], in0=t1[:],
                                            in1=posib.sl(sc), op=OP.add)
                    for j in range(NDC):
                        pt = psum((P, P))
                        nc.tensor.transpose(pt[:, :], t1[:, j * P:(j + 1) * P],
                                            ident[:])
                        nc.vector.tensor_copy(
                            enc.sl(j, b * S + sc * P, P), pt[:, :])

            dump_fm("enc0", enc)

            # ================= helpers =================
            def load_cols(src, l, n):
                t = pb.tile([P, n], f32, tag="b")
                nc.sync.dma_start(t[:], src[l])
                return t

            def ln_fm2(blks, w_t, b_t, dst, dst8=None):
                """Batched LN over the feature (partition-chunk) dim.

                blks: list of (X, c0_in); block bi writes dst cols
                [bi*S, (bi+1)*S). dst8: optional fp8 shadow of dst.
                """
                nb = len(blks)
                mu_t = pr.tile([nb, S], f32, tag="r", name=_nm("mu"))
                var_t = pr.tile([nb, S], f32, tag="r", name=_nm("va"))
                for bi, (X, c0_in) in enumerate(blks):
                    st_ps = psum2()
                    mu_ps = st_ps[0:1, 0:S]
                    sq_ps = st_ps[0:1, S:2 * S]
                    for dc in range(NDC):
                        xs = X.sl(dc, c0_in, S)
                        nc.tensor.matmul(mu_ps, ones_col[:], xs,
                                         start=(dc == 0), stop=(dc == NDC - 1))
                        sq = pw.tile([P, S], bf16, tag="w")
                        nc.scalar.activation(out=sq[:], in_=xs, func=AF.Square)
                        nc.tensor.matmul(sq_ps, ones_col[:], sq[:],
                                         start=(dc == 0), stop=(dc == NDC - 1))
                    if bi == 0:
                        mu_w, var_w = mu_t[0:1, :], var_t[0:1, :]
                    else:
                        mu_tmp = pr.tile([1, S], f32, tag="r", name=_nm("mt"))
                        var_tmp = pr.tile([1, S], f32, tag="r", name=_nm("vt"))
                        mu_w, var_w = mu_tmp[:], var_tmp[:]
                    nc.vector.tensor_scalar(
                        out=mu_w, in0=mu_ps,
                        scalar1=1.0 / D, scalar2=None, op0=OP.mult)
                    mu2 = pr.tile([1, S], f32, tag="r", name=_nm("m2"))
                    nc.vector.tensor_tensor(
                        out=mu2[:], in0=mu_w, in1=mu_w, op=OP.mult)
                    nc.vector.scalar_tensor_tensor(
                        out=var_w, in0=sq_ps,
                        scalar=1.0 / D, in1=mu2[:],
                        op0=OP.mult, op1=OP.subtract)
                    if bi > 0:
                        nc.sync.dma_start(mu_t[bi:bi + 1, :], mu_w)
                        nc.sync.dma_start(var_t[bi:bi + 1, :], var_w)
                sd_t = pr.tile([nb, S], f32, tag="r", name=_nm("sd"))
                nc.scalar.activation(out=sd_t[:, :], in_=var_t[:, :],
                                     func=AF.Sqrt, bias=eps_col[0:nb, :])
                g_r = pr.tile([nb, S], bf16, tag="r", name=_nm("gr"))
                nc.vector.reciprocal(out=g_r[:, :], in_=sd_t[:, :])
                c_r = pr.tile([nb, S], bf16, tag="r", name=_nm("cr"))
                nc.vector.tensor_tensor(out=c_r[:, :], in0=mu_t[:, :],
                                        in1=g_r[:, :], op=OP.mult)
                gc = []
                for bi in range(nb):
                    if bi == 0:
                        g0, c0 = g_r[0:1, :], c_r[0:1, :]
                    else:
                        g0 = pst.tile([1, S], bf16, tag="dn", bufs=4,
                                      name=_nm("g0"))
                        nc.sync.dma_start(g0[:], g_r[bi:bi + 1, :])
                        c0 = pst.tile([1, S], bf16, tag="dn", bufs=4,
                                      name=_nm("c0"))
                        nc.sync.dma_start(c0[:], c_r[bi:bi + 1, :])
                        g0, c0 = g0[:], c0[:]
                    gc_ps = psum2()
                    nc.tensor.matmul(gc_ps[:, 0:S], ones_row[:], g0,
                                     start=True, stop=True)
                    nc.tensor.matmul(gc_ps[:, S:2 * S], ones_row[:], c0,
                                     start=True, stop=True)
                    gc.append(gc_ps)
                for dc in range(NDC):
                    t2p = pw.tile([P, 2 * S], bf16, tag="wp", bufs=3)
                    for bi, (X, c0_in) in enumerate(blks):
                        t1 = pw.tile([P, S], bf16, tag="w")
                        nc.vector.tensor_tensor(
                            out=t1[:], in0=X.sl(dc, c0_in, S),
                            in1=gc[bi][:, 0:S], op=OP.mult)
                        nc.vector.tensor_tensor(
                            out=t2p[:, bi * S:(bi + 1) * S], in0=t1[:],
                            in1=gc[bi][:, S:2 * S], op=OP.subtract)
                    nc.scalar.activation(
                        out=dst.sl(dc, 0, 2 * S), in_=t2p[:],
                        func=AF.Identity,
                        scale=w_t[:, dc:dc + 1], bias=b_t[:, dc:dc + 1])
                    if dst8 is not None:
                        nc.vector.tensor_copy(dst8.sl(dc, 0, 2 * S),
                                              dst.sl(dc, 0, 2 * S))

            # ================= layers =================
            for l in range(n_layers):
                bqt = load_cols(bqc, l, NDC)
                bkt = load_cols(bkc, l, NDC)
                bfct = load_cols(bfcc, l, NDC)
                b1t = load_cols(b1c, l, NHC)
                b2t = load_cols(b2c, l, NDC)
                l1wt = load_cols(l1wc, l, NDC)
                l1bt = load_cols(l1bc, l, NDC)
                l2wt = load_cols(l2wc, l, NDC)
                l2bt = load_cols(l2bc, l, NDC)
                bvrep = pbv.tile([P, D], f32, tag="bv")
                nc.sync.dma_start(bvrep[:], bv[l].rearrange("(o d) -> o d", o=1)
                                  .to_broadcast((P, D)))

                # ---- q/k projections, both batches, weights once ----
                qk = {}
                for (nm, W, bt) in (("q", WqT, bqt), ("k", WkT, bkt)):
                    qk[nm] = FM(pu, NDC, T, bf16)
                    for mg in range(NDC // 2):
                        pps = [psum2(), psum2()]
                        for kk in range(NDC // 2):
                            wt = pwl.tile([P, 512], bf16, tag="wl")
                            wt3 = wt[:].rearrange("p (kk c) -> p kk c", kk=2)
                            nc.sync.dma_start(
                                wt3, W[l, 2 * kk * P:(2 * kk + 2) * P,
                                       mg * 256:(mg + 1) * 256]
                                .rearrange("(kk p) c -> p kk c", kk=2))
                            for ki in range(2):
                                k = 2 * kk + ki
                                for j in range(2):
                                    for b in range(BL):
                                        nc.tensor.matmul(
                                            pps[j][:, b * S:(b + 1) * S],
                                            wt[:, ki * 256 + j * P:
                                               ki * 256 + (j + 1) * P],
                                            enc.sl(k, b * S, S),
                                            start=(k == 0),
                                            stop=(k == NDC - 1))
                        for j in range(2):
                            m = mg * 2 + j
                            nc.scalar.activation(
                                out=qk[nm].sl(m), in_=pps[j][:, :],
                                func=AF.Identity, bias=bt[:, m:m + 1])

                if l == 0:
                    dump_fm("q0", qk["q"])
                    dump_fm("k0", qk["k"])

                # ---- v projection: token-major, 65 cols/head (65th = ones)
                def vproj(b):
                    vT = FM(pu, 4, H * 65, bf16)
                    for tcc in range(4):
                        u = vT.sl(tcc, 0, H * 65)
                        v3 = u.rearrange("p (h f) -> p h f", f=65)
                        nc.vector.memset(v3[:, :, 64:65], 1.0)
                    for n in range(2):
                        pps = [psum2(), psum2()]
                        for k in range(NDC):
                            wt = pwr.tile([P, 512], bf16, tag="wr")
                            nc.sync.dma_start(
                                wt[:], WvT[l, k * P:(k + 1) * P,
                                           n * 512:(n + 1) * 512])
                            for tcc in range(4):
                                nc.tensor.matmul(
                                    pps[tcc // 2][:, (tcc % 2) * S:
                                                  (tcc % 2 + 1) * S],
                                    enc.sl(k, b * S + tcc * P, P), wt[:],
                                    start=(k == 0), stop=(k == NDC - 1))
                        for tcc in range(4):
                            u = vT.sl(tcc, n * 8 * 65, 8 * 65)
                            dst3 = u.rearrange("p (h f) -> p h f", f=65)
                            nc.vector.tensor_tensor(
                                out=dst3[:, :, 0:64],
                                in0=pps[tcc // 2][:, (tcc % 2) * S:
                                                  (tcc % 2 + 1) * S]
                                .rearrange("p (h f) -> p h f", f=64),
                                in1=bvrep[:, n * 512:(n + 1) * 512]
                                .rearrange("p (h f) -> p h f", f=64),
                                op=OP.add)
                    return vT

                # ---- attention phase A: scores, exp, AV (+den via ones col)
                def attnA(b, vT):
                    qb, kb = qk["q"], qk["k"]
                    att = FM(pu, NDC, S, bf16)
                    denb = pst.tile([H, S], f32, tag="db", name=_nm("db"),
                                    bufs=2)
                    for h in range(H):
                        dc = h // 2
                        po = (h % 2) * DK
                        exps = []
                        for kcp in range(2):
                            sc2 = psum2()
                            for i in range(2):
                                kc = 2 * kcp + i
                                nc.tensor.matmul(
                                    sc2[:, i * S:(i + 1) * S],
                                    kb.sl(dc, b * S + kc * P, P,
                                          p0=po, np_=DK),
                                    qb.sl(dc, b * S, S, p0=po, np_=DK),
                                    start=True, stop=True)
                            ex = pex.tile([P, 2 * S], bf16, tag="ex")
                            nc.scalar.activation(out=ex[:], in_=sc2[:, :],
                                                 func=AF.Exp, scale=0.125)
                            exps.append(ex)
                        av_ps = psum((65, S))
                        for kc in range(4):
                            nc.tensor.matmul(
                                av_ps[:, :], vT.sl(kc, h * 65, 65),
                                exps[kc // 2][:, (kc % 2) * S:(kc % 2 + 1) * S],
                                start=(kc == 0), stop=(kc == 3))
                        dtmp = pst.tile([1, S], f32, tag="dn2", bufs=4,
                                        name=_nm("dt"))
                        nc.vector.tensor_copy(dtmp[:], av_ps[64:65, :])
                        nc.sync.dma_start(denb[h:h + 1, :], dtmp[:])
                        nc.vector.tensor_copy(
                            att.sl(dc, 0, S, p0=po, np_=DK), av_ps[0:64, :])
                    return att, denb

                def attn_recip(denb):
                    denr = pr.tile([H, S], bf16, tag="dr", name=_nm("dr"),
                                   bufs=2)
                    nc.vector.reciprocal(out=denr[:, :], in_=denb[:, :])
                    return denr

                # ---- attention phase C: normalize by 1/den
                def attnC(att, denr):
                    for dc in range(NDC):
                        rep_ps = psum((P, S))
                        for i in range(2):
                            h = 2 * dc + i
                            den0 = pst.tile([1, S], bf16, tag="dn", bufs=4,
                                            name=_nm("dn"))
                            nc.sync.dma_start(den0[:], denr[h:h + 1, :])
                            nc.tensor.matmul(rep_ps[i * DK:(i + 1) * DK, :],
                                             ones_row[:, 0:DK], den0[:],
                                             start=True, stop=True,
                                             tile_position=(0, i * DK),
                                             skip_group_check=True)
                        asl = att.sl(dc, 0, S)
                        nc.vector.tensor_tensor(out=asl, in0=asl,
                                                in1=rep_ps[:, :], op=OP.mult)

                # ---- fc + bias + residual for batch b ----
                def fc(b, att, C):
                    for mg in range(NDC // 2):
                        pp = psum2()
                        for kk in range(NDC // 2):
                            wt = pwl.tile([P, 512], bf16, tag="wl")
                            wt3 = wt[:].rearrange("p (kk c) -> p kk c", kk=2)
                            nc.sync.dma_start(
                                wt3, WfcT[l, 2 * kk * P:(2 * kk + 2) * P,
                                          mg * 256:(mg + 1) * 256]
                                .rearrange("(kk p) c -> p kk c", kk=2))
                            for ki in range(2):
                                k = 2 * kk + ki
                                for j in range(2):
                                    nc.tensor.matmul(
                                        pp[:, j * S:(j + 1) * S],
                                        wt[:, ki * 256 + j * P:
                                           ki * 256 + (j + 1) * P],
                                        att.sl(k), start=(k == 0),
                                        stop=(k == NDC - 1))
                        for j in range(2):
                            m = mg * 2 + j
                            nc.vector.scalar_tensor_tensor(
                                out=C.sl(m), in0=pp[:, j * S:(j + 1) * S],
                                scalar=bfct[:, m:m + 1],
                                in1=enc.sl(m, b * S, S),
                                op0=OP.add, op1=OP.add)

                vT0 = vproj(0)
                att0, den0 = attnA(0, vT0)
                vT1 = vproj(1)
                denr0 = attn_recip(den0)
                attnC(att0, denr0)
                att1, den1 = attnA(1, vT1)
                C = [FM(pu, NDC, S, bf16), FM(pu, NDC, S, bf16)]
                fc(0, att0, C[0])
                denr1 = attn_recip(den1)
                attnC(att1, denr1)
                fc(1, att1, C[1])
                if l == 0:
                    dump_fm("att0", att0)
                    dump_fm("c0", C[0])

                # ---- LN1 -> Dm (+ fp8 shadow for W1) ----
                Dm = FM(pu, NDC, 1024, bf16)
                Dm8 = FM(pu, NDC, 1024, f8)
                ln_fm2([(C[0], 0), (C[1], 0)], l1wt, l1bt, Dm, Dm8)

                # ---- FFN (both halves, weights once) ----
                hid = FM(pu, NHC, 1024, f8)
                for mg in range(NHC // 2):
                    pps = [psum2(), psum2()]
                    for kpp in range(NDC // 4):
                        wt = pwl.tile([P, 1024], f8, tag="wl")
                        wt4 = wt[:].rearrange("p (four c) -> p four c", four=4)
                        nc.sync.dma_start(
                            wt4, W1T[l, 4 * kpp * P:(4 * kpp + 4) * P,
                                     mg * 256:(mg + 1) * 256]
                            .rearrange("(four p) c -> p four c", four=4))
                        for i in range(2):
                            kp = 2 * kpp + i
                            for j in range(2):
                                for th in range(2):
                                    nc.tensor.matmul(
                                        pps[j][:, th * S:(th + 1) * S],
                                        wt4[:, 2 * i:2 * i + 2,
                                            j * P:(j + 1) * P],
                                        Dm8.sl2(kp, th * S, S),
                                        start=(kp == 0),
                                        stop=(kp == NDC // 2 - 1),
                                        perf_mode=mybir.MatmulPerfMode
                                        .DoubleRow)
                    for j in range(2):
                        m = mg * 2 + j
                        nc.scalar.activation(
                            out=hid.sl(m, 0, T), in_=pps[j][:, :],
                            func=AF.Relu, scale=1.0 / W2_SCALE,
                            bias=b1t[:, m:m + 1])
                E = FM(pu, NDC, 1024, bf16)
                for mg in range(NDC // 2):
                    pps = [psum2(), psum2()]
                    for kpp in range(NHC // 4):
                        wt = pwl.tile([P, 1024], f8, tag="wl")
                        wt4 = wt[:].rearrange("p (four c) -> p four c", four=4)
                        nc.sync.dma_start(
                            wt4, W2T[l, 4 * kpp * P:(4 * kpp + 4) * P,
                                     mg * 256:(mg + 1) * 256]
                            .rearrange("(four p) c -> p four c", four=4))
                        for i in range(2):
                            kp = 2 * kpp + i
                            for j in range(2):
                                for th in range(2):
                                    nc.tensor.matmul(
                                        pps[j][:, th * S:(th + 1) * S],
                                        wt4[:, 2 * i:2 * i + 2,
                                            j * P:(j + 1) * P],
                                        hid.sl2(kp, th * S, S),
                                        start=(kp == 0),
                                        stop=(kp == NHC // 2 - 1),
                                        perf_mode=mybir.MatmulPerfMode
                                        .DoubleRow)
                    for j in range(2):
                        m = mg * 2 + j
                        w2o = pw.tile([P, T], bf16, tag="wp", bufs=3)
                        nc.scalar.activation(
                            out=w2o[:], in_=pps[j][:, :],
                            func=AF.Identity, scale=1.0 / W2_SCALE,
                            bias=b2t[:, m:m + 1])
                        nc.vector.tensor_tensor(
                            out=E.sl(m, 0, T), in0=w2o[:],
                            in1=Dm.sl(m, 0, T), op=OP.add)

                # ---- LN2 -> next enc (fp8 for the final Wo GEMM) ----
                F = FM(pu, NDC, 1024, f8 if l == n_layers - 1 else bf16)
                ln_fm2([(E, 0), (E, S)], l2wt, l2bt, F)
                enc = F
                dump_fm(f"enc_l{l}", enc)

            # ================= final projection + log_softmax =================
            if do_final:
                for tg in range(2):
                    lgu = [pu.tile([P, 4096], f16, tag="u", name=_nm("lg"))
                           for _ in range(10)]

                    def lgsl(tcc, n, ncols):
                        fi = tcc * 10240 + n * 512
                        return lgu[fi // 4096][:, fi % 4096: fi % 4096 + ncols]

                    zaccs = [pst.tile([P, NOC // 2], f32, tag="z",
                                      name=_nm("za")) for _ in range(4)]
                    for n in range(NOC):
                        ncols = 512 if n < NOC - 1 else DOUT - (NOC - 1) * 512
                        borep = pb.tile([P, 512], bf16, tag="bon", bufs=4)
                        nc.sync.dma_start(
                            borep[:, :ncols],
                            bo[n * 512:n * 512 + ncols]
                            .rearrange("(o d) -> o d", o=1)
                            .to_broadcast((P, ncols)))
                        pps = [psum2(), psum2()]
                        for kpq in range(NDC // 4):
                            wt = pwr.tile([P, 2048], f8, tag="wr")
                            wt4 = wt[:, :4 * ncols].rearrange(
                                "p (four c) -> p four c", four=4)
                            nc.sync.dma_start(
                                wt4, WoT[4 * kpq * P:(4 * kpq + 4) * P,
                                         n * 512:n * 512 + ncols]
                                .rearrange("(four p) c -> p four c", four=4))
                            for i in range(2):
                                kp = 2 * kpq + i
                                for tcc in range(4):
                                    nc.tensor.matmul(
                                        pps[tcc // 2][:, (tcc % 2) * S:
                                                      (tcc % 2) * S + ncols],
                                        enc.sl2(kp, tg * S + tcc * P, P),
                                        wt4[:, 2 * i:2 * i + 2, :],
                                        start=(kp == 0),
                                        stop=(kp == NDC // 2 - 1),
                                        perf_mode=mybir.MatmulPerfMode
                                        .DoubleRow)
                        for tcc in range(4):
                            lsl = lgsl(tcc, n, ncols)
                            nc.vector.scalar_tensor_tensor(
                                out=lsl, in0=pps[tcc // 2][:, (tcc % 2) * S:
                                                           (tcc % 2) * S + ncols],
                                scalar=1.0 / W2_SCALE, in1=borep[:, :ncols],
                                op0=OP.mult, op1=OP.add)
                            if n % 2 == 1:
                                exs = pw.tile([P, 2 * S], f16, tag="wp",
                                              bufs=3)
                                nc.scalar.activation(
                                    out=exs[:, :512 + ncols],
                                    in_=lgsl(tcc, n - 1, 512 + ncols),
                                    func=AF.Exp,
                                    accum_out=zaccs[tcc][:, n // 2:n // 2 + 1])
                    for tcc in range(4):
                        z = pst.tile([P, 1], f32, tag="st")
                        nc.vector.reduce_sum(z[:], zaccs[tcc][:],
                                             axis=mybir.AxisListType.X)
                        lz = pst.tile([P, 1], f32, tag="st")
                        nc.scalar.activation(out=lz[:], in_=z[:], func=AF.Ln)
                        for n in range(NOC):
                            ncols = 512 if n < NOC - 1 else DOUT - (NOC - 1) * 512
                            nc.vector.tensor_scalar(
                                out=lgsl(tcc, n, ncols), in0=lgsl(tcc, n, ncols),
                                scalar1=lz[:], scalar2=None, op0=OP.subtract)
                        # batched output DMA: unit-aligned pieces
                        s0 = tcc * P
                        fi0 = tcc * 10240
                        off = 0
                        while off < DOUT:
                            fi = fi0 + off
                            u, ucol = fi // 4096, fi % 4096
                            ln_ = min(4096 - ucol, DOUT - off)
                            nc.sync.dma_start(
                                out[s0:s0 + P, tg, off:off + ln_],
                                lgu[u][:, ucol:ucol + ln_])
                            off += ln_
    nc.finalize()
    return nc


# ======================= host-side input prep =======================
def make_in_map(inp, core):
    """Build the per-core input dict from the full-problem input dict."""
    import ml_dtypes
    bf = ml_dtypes.bfloat16
    f8n = ml_dtypes.float8_e4m3
    f = np.float32
    c = np.ascontiguousarray
    b0 = core * BL
    m = {
        "xb": c(np.asarray(inp["x"], f)[:, b0:b0 + BL, :]),
        "rnd": c(np.asarray(inp["rnd"], f)[b0:b0 + BL, :]),
        "posib": c(np.asarray(inp["posi"], f)
                   + np.asarray(inp["ln0_b"], f)[None, :]),
        "ln0_w": c(np.asarray(inp["ln0_w"], f)),
        "bv": c(np.asarray(inp["bv"], f)),
        "bo": c(np.asarray(inp["bo"], f).astype(bf)),
    }
    tr = lambda a: c(np.asarray(a, f).transpose(0, 2, 1).astype(bf))
    m["WqT"] = tr(inp["Wq"])
    m["WkT"] = tr(inp["Wk"])
    m["WvT"] = tr(inp["Wv"])
    m["WfcT"] = tr(inp["Wfc"])
    m["W1T"] = c((np.asarray(inp["W1"], f).transpose(0, 2, 1)
                  * W2_SCALE).astype(f8n))
    m["W2T"] = c((np.asarray(inp["W2"], f).transpose(0, 2, 1)
                  * W2_SCALE).astype(f8n))
    m["WoT"] = c((np.asarray(inp["Wo"], f).T * W2_SCALE).astype(f8n))
    cols = lambda a, nch: c(np.asarray(a, f).reshape(L, nch, P).transpose(0, 2, 1))
    m["bqc"] = cols(inp["bq"], NDC)
    m["bkc"] = cols(inp["bk"], NDC)
    m["bfcc"] = cols(inp["bfc"], NDC)
    m["b1c"] = cols(inp["b1"], NHC)
    m["b2c"] = cols(inp["b2"], NDC)
    m["l1wc"] = cols(inp["ln1_w"], NDC)
    m["l1bc"] = cols(inp["ln1_b"], NDC)
    m["l2wc"] = cols(inp["ln2_w"], NDC)
    m["l2bc"] = cols(inp["ln2_b"], NDC)
    return m


def fm_to_np(arr, nch, ncols, dtype_bytes=4):
    """[n_units, 128, unit_cols] -> [nch*128, ncols]."""
    n_units = arr.shape[0]
    uw = arr.shape[2]
    cpu = uw // ncols
    out = np.zeros((nch * P, ncols), arr.dtype)
    for dc in range(nch):
        u = arr[dc // cpu]
        base = (dc % cpu) * ncols
        out[dc * P:(dc + 1) * P, :] = u[:, base:base + ncols]
    return out


# ======================= entry point =======================
_NC_CACHE = {}


def _get_nc(n_cores=8):
    if n_cores not in _NC_CACHE:
        _NC_CACHE[n_cores] = build(n_layers=L, do_final=True, dumps=(),
                                   n_cores=n_cores)
    return _NC_CACHE[n_cores]


def kernel(**inputs):
    """Full-input, full-output entry point. Shards batch across 8 cores."""
    from concourse.bass_utils import run_bass_kernel_spmd
    n_cores = 8
    nc = _get_nc(n_cores)
    inp = {k: np.asarray(v) for k, v in inputs.items()}
    in_maps = [make_in_map(inp, c) for c in range(n_cores)]
    res = run_bass_kernel_spmd(nc, in_maps, list(range(n_cores)))
    outs = [np.asarray(res.results[c]["out"], np.float32) for c in range(n_cores)]
    return np.concatenate(outs, axis=1)
